# revision 1
# baseline (speedup 1.0000x reference)
"""GraphTransformerLayer on 8 TRN2 NeuronCores (Bass/Tile).

Sharding: query/node dim N=2048 split into 8 shards of 256 rows.
Each core computes full K/V (replicated) + attention/FFN for its shard.
Edge bias is scattered on host into a dense per-core (H, N_keys, 256)
slab; softmax is unnormalized-exp (scores are bounded ~|1|) with the
denominator computed as an extra all-ones column of V.
"""

import sys

sys.path.insert(0, "/opt/trn_rl_repo")

import numpy as np

import concourse.bacc as bacc
import concourse.mybir as mybir
import concourse.tile as tile
from concourse.bass_utils import run_bass_kernel_spmd

N_CORES = 8
N = 2048
D = 256
H = 8
DK = 32
QS = N // N_CORES  # 256 query rows per core
H2 = 512
EPS = 1e-5

F32 = mybir.dt.float32
FR = mybir.dt.float32r
BF = mybir.dt.bfloat16





def build_kernel(use_fr=True):
    MT = F32
    AT = BF if use_fr else F32
    nc = bacc.Bacc("TRN2", target_bir_lowering=False, debug=False,
                   num_devices=N_CORES)

    d_hT = nc.dram_tensor("hT", [D, N], F32, kind="ExternalInput")
    d_hTs = nc.dram_tensor("hTs", [D, QS], F32, kind="ExternalInput")
    d_hres = nc.dram_tensor("hres", [QS, D], F32, kind="ExternalInput")
    d_biasT = nc.dram_tensor("biasT", [H, N, QS], F32, kind="ExternalInput")
    d_wq = nc.dram_tensor("wq", [D, D], F32, kind="ExternalInput")
    d_wk = nc.dram_tensor("wk", [D, D], F32, kind="ExternalInput")
    d_wv = nc.dram_tensor("wv", [D, 272], F32, kind="ExternalInput")
    d_bq = nc.dram_tensor("bq", [D, 1], F32, kind="ExternalInput")
    d_bk = nc.dram_tensor("bk", [D, 1], F32, kind="ExternalInput")
    d_bv = nc.dram_tensor("bv", [1, 272], F32, kind="ExternalInput")
    d_wo = nc.dram_tensor("wo", [D, D], F32, kind="ExternalInput")
    d_bo = nc.dram_tensor("bo", [1, D], F32, kind="ExternalInput")
    d_g1 = nc.dram_tensor("g1", [128, D], F32, kind="ExternalInput")
    d_be1 = nc.dram_tensor("be1", [128, D], F32, kind="ExternalInput")
    d_g2 = nc.dram_tensor("g2", [128, D], F32, kind="ExternalInput")
    d_be2 = nc.dram_tensor("be2", [128, D], F32, kind="ExternalInput")
    d_w1 = nc.dram_tensor("w1", [D, H2], F32, kind="ExternalInput")
    d_b1 = nc.dram_tensor("b1", [H2, 1], F32, kind="ExternalInput")
    d_w2 = nc.dram_tensor("w2", [H2, D], F32, kind="ExternalInput")
    d_b2 = nc.dram_tensor("b2", [D, 1], F32, kind="ExternalInput")
    d_id = nc.dram_tensor("ident", [128, 128], F32, kind="ExternalInput")
    d_out = nc.dram_tensor("out", [QS, D], F32, kind="ExternalOutput")

    with tile.TileContext(nc) as tc:
        import contextlib

        with contextlib.ExitStack() as ctx:
            wpool = ctx.enter_context(tc.tile_pool(name="weights", bufs=1))
            big = ctx.enter_context(tc.tile_pool(name="big", bufs=1))
            ptp = ctx.enter_context(tc.tile_pool(name="pt", bufs=2))
            bias_p = ctx.enter_context(tc.tile_pool(name="bias", bufs=6))
            sm = ctx.enter_context(tc.tile_pool(name="small", bufs=2))
            smk = ctx.enter_context(tc.tile_pool(name="smallk", bufs=1))
            ps_a = ctx.enter_context(
                tc.tile_pool(name="psA", bufs=2, space="PSUM"))
            ps_st = ctx.enter_context(
                tc.tile_pool(name="psST", bufs=3, space="PSUM"))
            ps_o = ctx.enter_context(
                tc.tile_pool(name="psO", bufs=2, space="PSUM"))

            # ---------- load weights / inputs ----------
            def load(pool, dram, shape, row0=0, col0=0, name=None, dt=F32):
                t = pool.tile(shape, dt, name=name or f"{dram.name}_sb_{row0}_{col0}")
                nc.sync.dma_start(
                    t[:], dram.ap()[row0:row0 + shape[0],
                                    col0:col0 + shape[1]])
                return t

            hT = [load(big, d_hT, [128, N], 128 * i) for i in range(2)]
            hTs = [load(big, d_hTs, [128, QS], 128 * i) for i in range(2)]
            hres = [load(big, d_hres, [128, D], 128 * i) for i in range(2)]
            wq = [load(wpool, d_wq, [128, D], 128 * i) for i in range(2)]
            wk = [load(wpool, d_wk, [128, D], 128 * i) for i in range(2)]
            wv = [load(wpool, d_wv, [128, 272], 128 * i) for i in range(2)]
            wo = [load(wpool, d_wo, [128, D], 128 * i) for i in range(2)]
            w1 = [load(wpool, d_w1, [128, H2], 128 * i) for i in range(2)]
            w2 = [load(wpool, d_w2, [128, D], 128 * i) for i in range(4)]
            bq = [load(wpool, d_bq, [128, 1], 128 * i) for i in range(2)]
            bk = [load(wpool, d_bk, [128, 1], 128 * i) for i in range(2)]
            b1 = [load(wpool, d_b1, [128, 1], 128 * i) for i in range(4)]
            b2 = [load(wpool, d_b2, [128, 1], 128 * i) for i in range(2)]
            bv = load(wpool, d_bv, [1, 272])
            bo = load(wpool, d_bo, [1, D])
            g1t = load(wpool, d_g1, [128, D])
            be1t = load(wpool, d_be1, [128, D])
            g2t = load(wpool, d_g2, [128, D])
            be2t = load(wpool, d_be2, [128, D])
            ident = load(wpool, d_id, [128, 128])
            ones = wpool.tile([1, 128], F32, name="ones")
            nc.vector.memset(ones[:], 1.0)
            zcol = wpool.tile([128, 1], F32, name="zcol")
            nc.vector.memset(zcol[:], 0.0)
            epscol = wpool.tile([128, 1], F32, name="epscol")
            nc.vector.memset(epscol[:], EPS)

            # ---------- projections ----------
            # QT[o, q] (2 tiles of 128): lhsT = wq chunk, rhs = hTs chunk
            QT = []
            for oc in range(2):
                ps = ps_a.tile([128, QS], F32, tag="psa", name="psq")
                for ic in range(2):
                    nc.tensor.matmul(
                        ps[:], (wq[ic][:, 128 * oc:128 * oc + 128]),
                        hTs[ic][:],
                        start=(ic == 0), stop=(ic == 1))
                t = big.tile([128, QS], AT, tag=f"QT{oc}", name=f"QT{oc}")
                nc.scalar.activation(t[:], ps[:],
                                     mybir.ActivationFunctionType.Identity,
                                     bias=bq[oc][:])
                QT.append(t)

            KT = [big.tile([128, N], AT, tag=f"KT{oc}", name=f"KT{oc}") for oc in range(2)]
            for oc in range(2):
                for fc in range(4):
                    ps = ps_a.tile([128, 512], F32, tag="psa", name="psk")
                    for ic in range(2):
                        nc.tensor.matmul(
                            ps[:],
                            (wk[ic][:, 128 * oc:128 * oc + 128]),
                            (hT[ic][:, 512 * fc:512 * fc + 512]),
                            start=(ic == 0), stop=(ic == 1))
                    nc.scalar.activation(
                        KT[oc][:, 512 * fc:512 * fc + 512], ps[:],
                        mybir.ActivationFunctionType.Identity, bias=bk[oc][:])

            # V natural (node, feat) augmented with per-head ones column:
            # v_sb[:, 264*c + 33*h + j]
            v_sb = big.tile([128, 16 * 272], AT, name="v_sb")
            for cchunk in range(16):
                ps = ps_a.tile([128, 272], F32, tag="psa", name="psv")
                for ic in range(2):
                    nc.tensor.matmul(
                        ps[:],
                        (hT[ic][:, 128 * cchunk:128 * cchunk + 128]),
                        wv[ic][:],
                        start=(ic == 0), stop=False)
                nc.tensor.matmul(ps[:], ones[:],
                                 bv[:],
                                 start=False, stop=True)
                nc.vector.tensor_copy(
                    v_sb[:, 272 * cchunk:272 * cchunk + 272], ps[:])

            # ---------- attention ----------
            o_nat = [big.tile([128, D], F32, tag=f"onat{qt}", name=f"onat{qt}")
                     for qt in range(2)]
            for h in range(8):
                tl, bp = h // 4, 32 * (h % 4)
                pt = ptp.tile([128, 16 * QS], AT, tag="pt", name="pt")
                for c in range(16):
                    ps = ps_st.tile([128, QS], F32, tag="pst", name="st_ps")
                    nc.tensor.matmul(
                        ps[:],
                        (KT[tl][bp:bp + 32, 128 * c:128 * c + 128]),
                        (QT[tl][bp:bp + 32, :]),
                        start=True, stop=True, tile_position=(bp, 0))
                    bt = bias_p.tile([128, QS], F32, tag="bias", name="bias_t")
                    nc.sync.dma_start(
                        bt[:], d_biasT.ap()[h, 128 * c:128 * c + 128, :])
                    nc.vector.tensor_add(
                        pt[:, QS * c:QS * c + QS], ps[:], bt[:])
                nc.scalar.activation(pt[:], pt[:],
                                     mybir.ActivationFunctionType.Exp,
                                     bias=zcol[:])
                for qt in range(2):
                    ops = ps_o.tile([128, 34], F32, tag="o", name="o_ps")
                    for c in range(16):
                        nc.tensor.matmul(
                            ops[:],
                            (
                                pt[:, QS * c + 128 * qt:QS * c + 128 * qt + 128]),
                            (
                                v_sb[:, 272 * c + 34 * h:272 * c + 34 * h + 34]),
                            start=(c == 0), stop=(c == 15))
                    rden = sm.tile([128, 1], F32, tag="rden", name="rden")
                    nc.vector.reciprocal(rden[:], ops[:, 32:33])
                    nc.vector.tensor_scalar_mul(
                        o_nat[qt][:, 32 * h:32 * h + 32], ops[:, 0:32],
                        rden[:])

            # ---------- output projection + residual + LN ----------
            OT = [big.tile([128, D], F32, tag=f"OT{fc}", name=f"OT{fc}") for fc in range(2)]
            for qt in range(2):
                for fc in range(2):
                    tps = ps_a.tile([128, 128], F32, tag="psa", name="tr_ps")
                    nc.tensor.transpose(
                        tps[:], o_nat[qt][:, 128 * fc:128 * fc + 128],
                        ident[:])
                    nc.vector.tensor_copy(
                        OT[fc][:, 128 * qt:128 * qt + 128], tps[:])

            def layer_norm(src_tiles, gamma, beta, out_tag):
                outs = []
                for qt in range(2):
                    x = src_tiles[qt]
                    ssum = sm.tile([128, 1], F32, tag="lnsum")
                    nc.vector.reduce_sum(ssum[:], x[:],
                                         axis=mybir.AxisListType.X)
                    negmean = sm.tile([128, 1], F32, tag="lnneg")
                    nc.scalar.mul(negmean[:], ssum[:], -1.0 / D)
                    xc = sm.tile([128, D], F32, tag="lnxc")
                    nc.scalar.activation(
                        xc[:], x[:], mybir.ActivationFunctionType.Identity,
                        bias=negmean[:])
                    scr = sm.tile([128, D], F32, tag="lnscr")
                    vs = sm.tile([128, 1], F32, tag="lnvs")
                    nc.scalar.activation(
                        scr[:], xc[:], mybir.ActivationFunctionType.Square,
                        bias=zcol[:], accum_out=vs[:])
                    st = sm.tile([128, 1], F32, tag="lnstd")
                    nc.scalar.activation(
                        st[:], vs[:], mybir.ActivationFunctionType.Sqrt,
                        bias=epscol[:], scale=1.0 / D)
                    r0 = sm.tile([128, 1], F32, tag="lnr0")
                    nc.vector.reciprocal(r0[:], st[:])
                    # one Newton step for rsqrt accuracy:
                    # r1 = r0*(1.5 - 0.5*v*r0^2), v = vs/D + eps
                    vv = sm.tile([128, 1], F32, tag="lnvv")
                    nc.vector.tensor_scalar(
                        vv[:], vs[:], 1.0 / D, EPS,
                        op0=mybir.AluOpType.mult, op1=mybir.AluOpType.add)
                    rr = sm.tile([128, 1], F32, tag="lnrr")
                    nc.vector.tensor_mul(rr[:], r0[:], r0[:])
                    va = sm.tile([128, 1], F32, tag="lnva")
                    nc.vector.tensor_mul(va[:], vv[:], rr[:])
                    cc = sm.tile([128, 1], F32, tag="lncc")
                    nc.vector.tensor_scalar(
                        cc[:], va[:], -0.5, 1.5,
                        op0=mybir.AluOpType.mult, op1=mybir.AluOpType.add)
                    r1 = sm.tile([128, 1], F32, tag="lnr1")
                    nc.vector.tensor_mul(r1[:], r0[:], cc[:])
                    yp = sm.tile([128, D], F32, tag="lnyp")
                    nc.vector.tensor_scalar_mul(yp[:], xc[:], r1[:])
                    yg = sm.tile([128, D], F32, tag=f"{out_tag}{qt}")
                    nc.vector.tensor_mul(yg[:], yp[:], gamma[:])
                    nc.vector.tensor_add(yg[:], yg[:], beta[:])
                    outs.append(yg)
                return outs

            xin = []
            for qt in range(2):
                aps = ps_a.tile([128, D], F32, tag="psa", name="att_ps")
                for ic in range(2):
                    nc.tensor.matmul(
                        aps[:],
                        (OT[ic][:, 128 * qt:128 * qt + 128]),
                        wo[ic][:],
                        start=(ic == 0), stop=False)
                nc.tensor.matmul(aps[:], ones[:],
                                 bo[:],
                                 start=False, stop=True)
                x = smk.tile([128, D], F32, tag=f"xin{qt}", name=f"xin{qt}")
                nc.vector.tensor_add(x[:], aps[:], hres[qt][:])
                xin.append(x)

            h1 = layer_norm(xin, g1t, be1t, "h1")
            # keep h1 tiles alive in smk pool (bufs=1, unique tags)
            h1k = []
            for qt in range(2):
                t = smk.tile([128, D], F32, tag=f"h1k{qt}", name=f"h1k{qt}")
                nc.vector.tensor_copy(t[:], h1[qt][:])
                h1k.append(t)
            fln = layer_norm(h1k, g2t, be2t, "fln")

            # ---------- FFN ----------
            fT = [smk.tile([128, D], F32, tag=f"fT{ic}", name=f"fT{ic}") for ic in range(2)]
            for qt in range(2):
                for fc in range(2):
                    tps = ps_a.tile([128, 128], F32, tag="psa", name="tr2_ps")
                    nc.tensor.transpose(
                        tps[:], fln[qt][:, 128 * fc:128 * fc + 128], ident[:])
                    nc.vector.tensor_copy(
                        fT[fc][:, 128 * qt:128 * qt + 128], tps[:])

            g1T = [smk.tile([128, QS], F32, tag=f"g1T{oc}", name=f"g1T{oc}") for oc in range(4)]
            for oc in range(4):
                ps = ps_st.tile([128, QS], F32, tag="pst", name="ffn1_ps")
                for ic in range(2):
                    nc.tensor.matmul(
                        ps[:],
                        (w1[ic][:, 128 * oc:128 * oc + 128]),
                        fT[ic][:],
                        start=(ic == 0), stop=(ic == 1))
                nc.scalar.activation(
                    g1T[oc][:], ps[:], mybir.ActivationFunctionType.Gelu,
                    bias=b1[oc][:])

            y2T = [smk.tile([128, QS], F32, tag=f"y2T{oc}", name=f"y2T{oc}") for oc in range(2)]
            for oc in range(2):
                ps = ps_st.tile([128, QS], F32, tag="pst", name="ffn2_ps")
                for ic in range(4):
                    nc.tensor.matmul(
                        ps[:],
                        (w2[ic][:, 128 * oc:128 * oc + 128]),
                        g1T[ic][:],
                        start=(ic == 0), stop=(ic == 3))
                nc.scalar.activation(
                    y2T[oc][:], ps[:], mybir.ActivationFunctionType.Identity,
                    bias=b2[oc][:])

            out_sb = [smk.tile([128, D], F32, tag=f"out{qt}", name=f"outsb{qt}")
                      for qt in range(2)]
            for qt in range(2):
                for fc in range(2):
                    tps = ps_a.tile([128, 128], F32, tag="psa", name="tr3_ps")
                    nc.tensor.transpose(
                        tps[:], y2T[fc][:, 128 * qt:128 * qt + 128], ident[:])
                    nc.vector.tensor_add(
                        out_sb[qt][:, 128 * fc:128 * fc + 128],
                        h1k[qt][:, 128 * fc:128 * fc + 128], tps[:])
                nc.sync.dma_start(d_out.ap()[128 * qt:128 * qt + 128, :],
                                  out_sb[qt][:])

    nc.compile()
    return nc


_CACHE = {}
USE_FR = True


def _get_nc(use_fr=True):
    if use_fr not in _CACHE:
        _CACHE[use_fr] = build_kernel(use_fr)
    return _CACHE[use_fr]


def kernel(**inputs):
    h = np.asarray(inputs["h"], np.float32)
    edge_attr = np.asarray(inputs["edge_attr"], np.float32)
    edge_index = np.asarray(inputs["edge_index"])
    Wq, bq = np.asarray(inputs["Wq"], np.float32), np.asarray(inputs["bq"], np.float32)
    Wk, bk = np.asarray(inputs["Wk"], np.float32), np.asarray(inputs["bk"], np.float32)
    Wv, bv = np.asarray(inputs["Wv"], np.float32), np.asarray(inputs["bv"], np.float32)
    Wo, bo = np.asarray(inputs["Wo"], np.float32), np.asarray(inputs["bo"], np.float32)
    We, be = np.asarray(inputs["We"], np.float32), np.asarray(inputs["be"], np.float32)
    ln1_g, ln1_b = np.asarray(inputs["ln1_g"], np.float32), np.asarray(inputs["ln1_b"], np.float32)
    fln_g, fln_b = np.asarray(inputs["fln_g"], np.float32), np.asarray(inputs["fln_b"], np.float32)
    W1, b1 = np.asarray(inputs["W1"], np.float32), np.asarray(inputs["b1"], np.float32)
    W2, b2 = np.asarray(inputs["W2"], np.float32), np.asarray(inputs["b2"], np.float32)

    scale = 1.0 / np.sqrt(np.float32(DK))
    eb = edge_attr @ We + be  # (E, H)

    hT = np.ascontiguousarray(h.T)  # (D, N)
    wv_aug = np.zeros((D, 272), np.float32)
    bv_aug = np.zeros((1, 272), np.float32)
    for hh in range(H):
        wv_aug[:, 34 * hh:34 * hh + 32] = Wv[:, 32 * hh:32 * hh + 32]
        bv_aug[0, 34 * hh:34 * hh + 32] = bv[32 * hh:32 * hh + 32]
        bv_aug[0, 34 * hh + 32] = 1.0

    common = {
        "hT": hT,
        "wq": (Wq * scale).astype(np.float32),
        "wk": Wk, "wv": wv_aug,
        "bq": (bq * scale).reshape(D, 1).astype(np.float32),
        "bk": bk.reshape(D, 1), "bv": bv_aug,
        "wo": Wo, "bo": bo.reshape(1, D),
        "g1": np.tile(ln1_g, (128, 1)), "be1": np.tile(ln1_b, (128, 1)),
        "g2": np.tile(fln_g, (128, 1)), "be2": np.tile(fln_b, (128, 1)),
        "w1": W1, "b1": b1.reshape(H2, 1),
        "w2": W2, "b2": b2.reshape(D, 1),
        "ident": np.eye(128, dtype=np.float32),
    }

    src = edge_index[0].astype(np.int64)
    dst = edge_index[1].astype(np.int64)
    in_maps = []
    for c in range(N_CORES):
        r0 = c * QS
        m = dict(common)
        m["hTs"] = np.ascontiguousarray(hT[:, r0:r0 + QS])
        m["hres"] = np.ascontiguousarray(h[r0:r0 + QS])
        biasT = np.zeros((H, N, QS), np.float32)
        sel = (src >= r0) & (src < r0 + QS)
        biasT[:, dst[sel], src[sel] - r0] = eb[sel].T
        m["biasT"] = biasT
        in_maps.append(m)

    nc = _get_nc(use_fr=USE_FR)
    res = run_bass_kernel_spmd(nc, in_maps, core_ids=list(range(N_CORES)))
    out = np.concatenate([res.results[c]["out"] for c in range(N_CORES)],
                         axis=0)
    return out.astype(np.float32)



# revision 14
# speedup vs baseline: 2.0703x; 2.0703x over previous
"""GraphTransformerLayer on 8 TRN2 NeuronCores (Bass/Tile).

Sharding: query/node dim N=2048 split into 8 shards of 256 rows; each core
holds replicated K/V for all 2048 keys plus its 256-query shard.

Design (v2):
- All matmuls in bf16 (fp32 runs at 1/4 PE rate); psum accumulation f32.
- Edge bias handled as a dense per-core fp8 slab [16][128 keys, 8 heads,
  256 queries] that is added into the score PSUM by seeding each psum
  accumulation group with an identity-matmul (start=True) before the
  K^T Q score matmuls (start=False, stop=True) land on top.
- Scores processed chunk-major: per key-chunk c (128 keys), per half g
  (4 heads), one [128, 4x256] psum tile -> single Exp activation psum->SBUF
  bf16. Unnormalized-softmax denominator comes from an extra all-ones
  column per head in V (attn @ [V|1]).
- attnV accumulates into persistent [128, 8, 33] psum tiles per query half.
- Epilogue (out-proj, LN, FFN) split per query half to shorten the tail.
"""

import sys

sys.path.insert(0, "/opt/trn_rl_repo")

import numpy as np

import concourse.bacc as bacc
import concourse.mybir as mybir
import concourse.tile as tile
from concourse.bass_utils import run_bass_kernel_spmd

N_CORES = 8
N = 2048
D = 256
H = 8
DK = 32
QS = 256
H2 = 512
EPS = 1e-5
NCH = 16  # key chunks of 128

F32 = mybir.dt.float32
BF = mybir.dt.bfloat16
F8 = mybir.dt.float8e4

AF = mybir.ActivationFunctionType
ALU = mybir.AluOpType
AX = mybir.AxisListType


def build_kernel():
    import os
    KCUT = int(os.environ.get("KCUT", "99"))
    nc = bacc.Bacc("TRN2", target_bir_lowering=False, debug=False,
                   num_devices=N_CORES)

    d_hT = nc.dram_tensor("hT", [D, N], BF, kind="ExternalInput")
    d_hTs = nc.dram_tensor("hTs", [D, QS], BF, kind="ExternalInput")
    d_hres = nc.dram_tensor("hres", [QS, D], BF, kind="ExternalInput")
    d_wq = nc.dram_tensor("wq", [D, D], BF, kind="ExternalInput")
    d_wk = nc.dram_tensor("wk", [D, D], BF, kind="ExternalInput")
    d_wv = nc.dram_tensor("wv", [D, D], BF, kind="ExternalInput")
    d_wo = nc.dram_tensor("wo", [D, D], BF, kind="ExternalInput")
    d_w1 = nc.dram_tensor("w1", [D, H2], BF, kind="ExternalInput")
    d_w2 = nc.dram_tensor("w2", [H2, D], BF, kind="ExternalInput")
    d_cols = nc.dram_tensor("cols", [128, 12], F32, kind="ExternalInput")
    d_bo = nc.dram_tensor("bo_eff", [1, D], BF, kind="ExternalInput")
    d_lng = nc.dram_tensor("lng", [128, 4 * D], BF, kind="ExternalInput")
    d_bias = nc.dram_tensor("bias8", [NCH, 128, H, QS], F8,
                            kind="ExternalInput")
    d_id8 = nc.dram_tensor("ident8", [128, 128], F8, kind="ExternalInput")
    d_idb = nc.dram_tensor("identb", [128, 128], BF, kind="ExternalInput")
    d_out = nc.dram_tensor("out", [QS, D], F32, kind="ExternalOutput")

    with tile.TileContext(nc) as tc:
        import contextlib

        with contextlib.ExitStack() as ctx:
            wp = ctx.enter_context(tc.tile_pool(name="w", bufs=1))
            bpool = ctx.enter_context(tc.tile_pool(name="bias", bufs=4))
            ptp = ctx.enter_context(tc.tile_pool(name="pt", bufs=5))
            sm = ctx.enter_context(tc.tile_pool(name="sm", bufs=2))
            ps_sc = ctx.enter_context(
                tc.tile_pool(name="psc", bufs=2, space="PSUM"))
            ps_at = ctx.enter_context(
                tc.tile_pool(name="pat", bufs=1, space="PSUM"))
            ps_ms = ctx.enter_context(
                tc.tile_pool(name="pms", bufs=2, space="PSUM"))

            def load(pool, dram, shape, name, dt, r0=0):
                t = pool.tile(shape, dt, name=name, tag=name)
                nc.sync.dma_start(
                    t[:], dram.ap()[r0:r0 + shape[0], 0:shape[1]])
                return t

            hT = [load(wp, d_hT, [128, N], f"hT{i}", BF, 128 * i)
                  for i in range(2)]
            hTs = [load(wp, d_hTs, [128, QS], f"hTs{i}", BF, 128 * i)
                   for i in range(2)]
            hres = [load(wp, d_hres, [128, D], f"hres{i}", BF, 128 * i)
                    for i in range(2)]
            wq = [load(wp, d_wq, [128, D], f"wq{i}", BF, 128 * i)
                  for i in range(2)]
            wk = [load(wp, d_wk, [128, D], f"wk{i}", BF, 128 * i)
                  for i in range(2)]
            wv = [load(wp, d_wv, [128, D], f"wv{i}", BF, 128 * i)
                  for i in range(2)]
            wo = [load(wp, d_wo, [128, D], f"wo{i}", BF, 128 * i)
                  for i in range(2)]
            w1 = [load(wp, d_w1, [128, H2], f"w1{i}", BF, 128 * i)
                  for i in range(2)]
            w2 = [load(wp, d_w2, [128, D], f"w2{i}", BF, 128 * i)
                  for i in range(4)]
            cols = load(wp, d_cols, [128, 12], "cols", F32)
            bo = load(wp, d_bo, [1, D], "bo", BF)
            lng = load(wp, d_lng, [128, 4 * D], "lng", BF)
            id8 = load(wp, d_id8, [128, 128], "id8", F8)
            idb = load(wp, d_idb, [128, 128], "idb", BF)

            bq = [cols[:, 0:1], cols[:, 1:2]]
            bk = [cols[:, 2:3], cols[:, 3:4]]
            b1c = [cols[:, 4 + i:5 + i] for i in range(4)]
            b2c = [cols[:, 8:9], cols[:, 9:10]]
            zcol = cols[:, 10:11]
            epscol = cols[:, 11:12]

            ones = wp.tile([1, 128], BF, name="ones", tag="ones")
            nc.vector.memset(ones[:], 1.0)

            # V with per-head all-ones denominator column (col 32 of 33)
            v_sb = wp.tile([128, NCH, H, 33], BF, name="v_sb", tag="v_sb")
            nc.vector.memset(v_sb[:, :, :, 32:33], 1.0)

            # bias slab tiles (fp8), prefetched rotating
            bias_tiles = {}

            def emit_bias_dma(c):
                t = bpool.tile([128, H, QS], F8, tag="bias", name=f"bias{c}")
                nc.sync.dma_start(t[:], d_bias.ap()[c])
                bias_tiles[c] = t

            # ---------- Q projection ----------
            QT = []
            for oc in range(2):
                ps = ps_ms.tile([128, 512], F32, tag="pms", name="psq")
                for ic in range(2):
                    nc.tensor.matmul(
                        ps[:, 0:QS], wq[ic][:, 128 * oc:128 * oc + 128],
                        hTs[ic][:], start=(ic == 0), stop=(ic == 1))
                t = wp.tile([128, QS], BF, name=f"QT{oc}", tag=f"QT{oc}")
                nc.vector.tensor_scalar_add(t[:], ps[:, 0:QS], bq[oc])
                QT.append(t)

            KT = [wp.tile([128, N], BF, name=f"KT{i}", tag=f"KT{i}")
                  for i in range(2)]
            att = [ps_at.tile([128, H, 33], F32, tag=f"att{qt}",
                              name=f"att{qt}") for qt in range(2)]
            for qt in range(2):
                nc.vector.memset(att[qt][:], 1.0 if KCUT < 4 else 0.0)
            pt_tiles = {}

            def emit_kproj(fc):
                for oc in range(2):
                    ps = ps_ms.tile([128, 512], F32, tag="pms", name="psk")
                    for ic in range(2):
                        nc.tensor.matmul(
                            ps[:], wk[ic][:, 128 * oc:128 * oc + 128],
                            hT[ic][:, 512 * fc:512 * fc + 512],
                            start=(ic == 0), stop=(ic == 1))
                    nc.vector.tensor_scalar_add(
                        KT[oc][:, 512 * fc:512 * fc + 512], ps[:], bk[oc])

            def emit_vproj(c):
                ps = ps_ms.tile([128, 512], F32, tag="pms", name="psv")
                for ic in range(2):
                    nc.tensor.matmul(
                        ps[:, 0:D], hT[ic][:, 128 * c:128 * c + 128],
                        wv[ic][:], start=(ic == 0), stop=(ic == 1))
                nc.vector.tensor_copy(
                    v_sb[:, c, :, 0:32],
                    ps[:, 0:D].rearrange("p (h e) -> p h e", h=H))

            # Head processing order: pairs sharing the same partition-offset
            # quadrant (bp) land in the same 2KB psum zero region — mixing
            # different lhsT/rhs partition offsets in one zero region hangs
            # the PE on hardware.
            ORDER = [0, 4, 1, 5, 2, 6, 3, 7]

            def emit_chunk(c):
                pt = ptp.tile([128, 2, 4, QS], BF, tag="pt", name=f"pt{c}")
                pt_tiles[c] = pt
                if KCUT < 2:
                    nc.vector.memset(pt[:], 1.0)
                    bias_tiles.pop(c)
                    return
                bt = bias_tiles.pop(c)
                for g in range(2):
                    ps = ps_sc.tile([128, 4, QS], F32, tag="sc",
                                    name=f"sc{c}_{g}")
                    if KCUT >= 3:
                        for s in range(2):
                            nc.tensor.matmul(
                                ps[:, 2 * s:2 * s + 2, :], id8[:],
                                bt[:, 4 * g + 2 * s:4 * g + 2 * s + 2, :],
                                start=True, stop=False, skip_group_check=True)
                    for hh in range(4):
                        h = ORDER[4 * g + hh]
                        bp = 32 * (h % 4)
                        nc.tensor.matmul(
                            ps[:, hh, :],
                            KT[h // 4][bp:bp + 32, 128 * c:128 * c + 128],
                            QT[h // 4][bp:bp + 32, :],
                            start=(KCUT < 3 and hh % 2 == 0),
                            stop=True, tile_position=(bp, 0),
                            skip_group_check=True)
                    nc.scalar.activation(pt[:, g], ps[:], AF.Exp, bias=zcol)

            def emit_attnv(c):
                pt = pt_tiles.pop(c)
                if KCUT < 4:
                    return
                for qt in range(2):
                    for g in range(2):
                        for hh in range(4):
                            h = ORDER[4 * g + hh]
                            nc.tensor.matmul(
                                att[qt][:, h, :],
                                pt[:, g, hh, 128 * qt:128 * qt + 128],
                                v_sb[:, c, h, :],
                                start=False, stop=(c == NCH - 1),
                                skip_group_check=True)

            # ---------- main pipeline emission ----------
            emit_bias_dma(0)
            emit_bias_dma(1)

            def emit_c(c):
                if c + 2 < NCH:
                    emit_bias_dma(c + 2)
                emit_chunk(c)
                if c >= 2:
                    emit_attnv(c - 2)

            emit_kproj(0)
            for cc in range(4):
                emit_vproj(cc)
            emit_c(0)
            emit_c(1)
            emit_kproj(1)
            for cc in range(4, 8):
                emit_vproj(cc)
            emit_c(2)
            emit_c(3)
            emit_kproj(2)
            for cc in range(8, 12):
                emit_vproj(cc)
            emit_c(4)
            emit_c(5)
            emit_kproj(3)
            for cc in range(12, 16):
                emit_vproj(cc)
            for c in range(6, NCH):
                emit_c(c)
            emit_attnv(NCH - 2)
            emit_attnv(NCH - 1)

            # ---------- epilogue ----------
            o_nat = [wp.tile([128, D], BF, name=f"onat{qt}", tag=f"onat{qt}")
                     for qt in range(2)]
            OT = [wp.tile([128, D], BF, name=f"OT{fc}", tag=f"OT{fc}")
                  for fc in range(2)]
            fT = [wp.tile([128, D], BF, name=f"fT{fc}", tag=f"fT{fc}")
                  for fc in range(2)]
            g1T = [wp.tile([128, QS], BF, name=f"g1T{oc}", tag=f"g1T{oc}")
                   for oc in range(4)]
            y2T = [wp.tile([128, QS], BF, name=f"y2T{oc}", tag=f"y2T{oc}")
                   for oc in range(2)]
            h1 = [wp.tile([128, D], F32, name=f"h1_{qt}", tag=f"h1_{qt}")
                  for qt in range(2)]
            out_sb = [wp.tile([128, D], F32, name=f"osb{qt}", tag=f"osb{qt}")
                      for qt in range(2)]

            def layer_norm(x_ap, g_ap, b_ap, out_ap):
                ss = sm.tile([128, 1], F32, tag="lns", name="ss")
                nc.vector.reduce_sum(ss[:], x_ap, axis=AX.X)
                nm = sm.tile([128, 1], F32, tag="lnm", name="nm")
                nc.vector.tensor_scalar_mul(nm[:], ss[:], -1.0 / D)
                xc = sm.tile([128, D], F32, tag="lnxc", name="xc")
                nc.vector.tensor_scalar_add(xc[:], x_ap, nm[:])
                scr = sm.tile([128, D], F32, tag="lnscr", name="scr")
                vs = sm.tile([128, 1], F32, tag="lnvs", name="vs")
                nc.vector.tensor_mul(scr[:], xc[:], xc[:])
                nc.vector.reduce_sum(vs[:], scr[:], axis=AX.X)
                st = sm.tile([128, 1], F32, tag="lnst", name="st")
                nc.scalar.activation(st[:], vs[:], AF.Sqrt, bias=epscol,
                                     scale=1.0 / D)
                r0 = sm.tile([128, 1], F32, tag="lnr0", name="r0")
                nc.vector.reciprocal(r0[:], st[:])
                yp = sm.tile([128, D], F32, tag="lnyp", name="yp")
                nc.vector.tensor_scalar_mul(yp[:], xc[:], r0[:])
                yg = sm.tile([128, D], F32, tag="lnyg", name="yg")
                nc.vector.tensor_mul(yg[:], yp[:], g_ap)
                nc.vector.tensor_add(out_ap, yg[:], b_ap)

            EP = KCUT if 10 <= KCUT <= 14 else 14
            for qt in range(2):
                if EP < 11:
                    osb = out_sb[qt]
                    nc.vector.tensor_copy(osb[:], hres[qt][:])
                    nc.sync.dma_start(d_out.ap()[128 * qt:128 * qt + 128, :],
                                      osb[:])
                    continue
                rdt = sm.tile([128, H, 1], F32, tag="rd", name=f"rd{qt}")
                nc.vector.reciprocal(rdt[:], att[qt][:, :, 32:33])
                for h in range(8):
                    nc.vector.tensor_scalar_mul(
                        o_nat[qt][:, 32 * h:32 * h + 32],
                        att[qt][:, h, 0:32], rdt[:, h])
                for fc in range(2):
                    tp = ps_ms.tile([128, 512], BF, tag="pms", name="trp")
                    nc.tensor.transpose(
                        tp[:, 0:128], o_nat[qt][:, 128 * fc:128 * fc + 128],
                        idb[:])
                    nc.vector.tensor_copy(
                        OT[fc][:, 128 * qt:128 * qt + 128], tp[:, 0:128])

                if EP < 12:
                    osb = out_sb[qt]
                    nc.vector.tensor_copy(osb[:, 0:128], OT[0][:, 128 * qt:128 * qt + 128])
                    nc.vector.tensor_copy(osb[:, 128:256], OT[1][:, 128 * qt:128 * qt + 128])
                    nc.sync.dma_start(d_out.ap()[128 * qt:128 * qt + 128, :],
                                      osb[:])
                    continue
                aps = ps_ms.tile([128, 512], F32, tag="pms", name="apo")
                for ic in range(2):
                    nc.tensor.matmul(
                        aps[:, 0:D], OT[ic][:, 128 * qt:128 * qt + 128],
                        wo[ic][:], start=(ic == 0), stop=False)
                nc.tensor.matmul(aps[:, 0:D], ones[:], bo[:],
                                 start=False, stop=True)
                xin = sm.tile([128, D], F32, tag="xin", name=f"xin{qt}")
                nc.vector.tensor_add(xin[:], aps[:, 0:D], hres[qt][:])

                fln = sm.tile([128, D], BF, tag="fln", name=f"fln{qt}")
                if EP < 13:
                    nc.vector.tensor_copy(h1[qt][:], xin[:])
                    nc.vector.tensor_copy(fln[:], xin[:])
                else:
                    layer_norm(xin[:], lng[:, 0:D], lng[:, D:2 * D], h1[qt][:])
                    layer_norm(h1[qt][:], lng[:, 2 * D:3 * D],
                               lng[:, 3 * D:4 * D], fln[:])

                if EP < 14:
                    nc.vector.tensor_copy(out_sb[qt][:], h1[qt][:])
                    nc.sync.dma_start(d_out.ap()[128 * qt:128 * qt + 128, :],
                                      out_sb[qt][:])
                    continue
                for fc in range(2):
                    tp2 = ps_ms.tile([128, 512], BF, tag="pms", name="trf")
                    nc.tensor.transpose(
                        tp2[:, 0:128], fln[:, 128 * fc:128 * fc + 128],
                        idb[:])
                    nc.vector.tensor_copy(
                        fT[fc][:, 128 * qt:128 * qt + 128], tp2[:, 0:128])

                for oc in range(4):
                    fp = ps_ms.tile([128, 512], F32, tag="pms", name="ps1")
                    for ic in range(2):
                        nc.tensor.matmul(
                            fp[:, 0:128],
                            w1[ic][:, 128 * oc:128 * oc + 128],
                            fT[ic][:, 128 * qt:128 * qt + 128],
                            start=(ic == 0), stop=(ic == 1))
                    nc.scalar.activation(
                        g1T[oc][:, 128 * qt:128 * qt + 128], fp[:, 0:128],
                        AF.Gelu, bias=b1c[oc])
                for oc in range(2):
                    fp = ps_ms.tile([128, 512], F32, tag="pms", name="ps2")
                    for ic in range(4):
                        nc.tensor.matmul(
                            fp[:, 0:128],
                            w2[ic][:, 128 * oc:128 * oc + 128],
                            g1T[ic][:, 128 * qt:128 * qt + 128],
                            start=(ic == 0), stop=(ic == 3))
                    nc.vector.tensor_scalar_add(
                        y2T[oc][:, 128 * qt:128 * qt + 128], fp[:, 0:128],
                        b2c[oc])
                for fc in range(2):
                    tp3 = ps_ms.tile([128, 512], BF, tag="pms", name="trn")
                    nc.tensor.transpose(
                        tp3[:, 0:128], y2T[fc][:, 128 * qt:128 * qt + 128],
                        idb[:])
                    nc.vector.tensor_add(
                        out_sb[qt][:, 128 * fc:128 * fc + 128],
                        h1[qt][:, 128 * fc:128 * fc + 128], tp3[:, 0:128])
                nc.sync.dma_start(d_out.ap()[128 * qt:128 * qt + 128, :],
                                  out_sb[qt][:])

    nc.compile()
    return nc


_CACHE = {}
USE_FR = True


def _get_nc(use_fr=True):
    if use_fr not in _CACHE:
        _CACHE[use_fr] = build_kernel()
    return _CACHE[use_fr]


def kernel(**inputs):
    import ml_dtypes

    bf = ml_dtypes.bfloat16
    f8 = ml_dtypes.float8_e4m3fn

    h = np.asarray(inputs["h"], np.float32)
    edge_attr = np.asarray(inputs["edge_attr"], np.float32)
    edge_index = np.asarray(inputs["edge_index"])
    Wq, bq = np.asarray(inputs["Wq"], np.float32), np.asarray(inputs["bq"], np.float32)
    Wk, bk = np.asarray(inputs["Wk"], np.float32), np.asarray(inputs["bk"], np.float32)
    Wv, bv = np.asarray(inputs["Wv"], np.float32), np.asarray(inputs["bv"], np.float32)
    Wo, bo = np.asarray(inputs["Wo"], np.float32), np.asarray(inputs["bo"], np.float32)
    We, be = np.asarray(inputs["We"], np.float32), np.asarray(inputs["be"], np.float32)
    ln1_g, ln1_b = np.asarray(inputs["ln1_g"], np.float32), np.asarray(inputs["ln1_b"], np.float32)
    fln_g, fln_b = np.asarray(inputs["fln_g"], np.float32), np.asarray(inputs["fln_b"], np.float32)
    W1, b1 = np.asarray(inputs["W1"], np.float32), np.asarray(inputs["b1"], np.float32)
    W2, b2 = np.asarray(inputs["W2"], np.float32), np.asarray(inputs["b2"], np.float32)

    scale = 1.0 / np.sqrt(np.float32(DK))
    eb = edge_attr @ We + be  # (E, H)
    hT = np.ascontiguousarray(h.T)

    cols = np.zeros((128, 12), np.float32)
    cols[:, 0] = bq[0:128] * scale
    cols[:, 1] = bq[128:256] * scale
    cols[:, 2] = bk[0:128]
    cols[:, 3] = bk[128:256]
    for i in range(4):
        cols[:, 4 + i] = b1[128 * i:128 * i + 128]
    cols[:, 8] = b2[0:128]
    cols[:, 9] = b2[128:256]
    cols[:, 11] = EPS

    bo_eff = (bv @ Wo + bo).reshape(1, D)
    lng = np.concatenate(
        [np.tile(x.reshape(1, D), (128, 1))
         for x in [ln1_g, ln1_b, fln_g, fln_b]], axis=1)

    common = {
        "hT": hT.astype(bf),
        "wq": (Wq * scale).astype(bf),
        "wk": Wk.astype(bf),
        "wv": Wv.astype(bf),
        "wo": Wo.astype(bf),
        "w1": W1.astype(bf),
        "w2": W2.astype(bf),
        "cols": cols,
        "bo_eff": bo_eff.astype(bf),
        "lng": lng.astype(bf),
        "ident8": np.eye(128, dtype=np.float32).astype(f8),
        "identb": np.eye(128, dtype=np.float32).astype(bf),
    }

    src = edge_index[0].astype(np.int64)
    dst = edge_index[1].astype(np.int64)
    in_maps = []
    for c in range(N_CORES):
        r0 = c * QS
        m = dict(common)
        m["hTs"] = np.ascontiguousarray(hT[:, r0:r0 + QS]).astype(bf)
        m["hres"] = np.ascontiguousarray(h[r0:r0 + QS]).astype(bf)
        sel = (src >= r0) & (src < r0 + QS)
        slab = np.zeros((N, H, QS), np.float32)
        slab[dst[sel], :, src[sel] - r0] = eb[sel]
        slab = slab[:, [0, 4, 1, 5, 2, 6, 3, 7], :]  # head slot order
        m["bias8"] = np.ascontiguousarray(
            slab.reshape(NCH, 128, H, QS)).astype(f8)
        in_maps.append(m)

    nc = _get_nc(USE_FR)
    res = run_bass_kernel_spmd(nc, in_maps, core_ids=list(range(N_CORES)))
    out = np.concatenate([res.results[cc]["out"] for cc in range(N_CORES)],
                         axis=0)
    return out.astype(np.float32)


# revision 15
# speedup vs baseline: 2.4699x; 1.1930x over previous
"""GraphTransformerLayer on 8 TRN2 NeuronCores (Bass/Tile).

Sharding: query/node dim N=2048 split into 8 shards of 256 rows; each core
holds replicated K/V for all 2048 keys plus its 256-query shard.

Design (v3):
- All matmuls bf16 (fp32 runs at 1/4 PE rate); psum accumulation f32.
- Edge bias: dense per-core fp8 slab [16][128 keys, 8 heads, 256 queries],
  added into the score PSUM by seeding each accumulation group with an
  fp8 identity-matmul (start=True) before the K^T Q score matmuls
  (start=False, stop=True) accumulate on top.
- Heads are processed in the order [0,4,1,5,2,6,3,7] so that the two heads
  sharing one 2KB psum zero-region use the same PE quadrant row offset —
  mixing different sub-128 partition offsets in one zero region hangs the
  PE on hardware.
- Scores chunk-major: per key-chunk c (128 keys) and half g (4 heads), one
  [128, 4x256] psum tile -> one Exp activation psum->SBUF bf16 (pt).
  Unnormalized softmax; denominator comes from an all-ones column per head
  appended to V (attn @ [V|1]).
- attnV accumulates into memset-seeded persistent [128, 8, 33] psum tiles
  per query half (start=False throughout).
- V projection is spread across the chunk loop as PE filler; epilogue is
  stage-interleaved across the two query halves with activation-table
  switches batched (Exp -> Sqrt -> Gelu).
"""

import sys

sys.path.insert(0, "/opt/trn_rl_repo")

import numpy as np

import concourse.bacc as bacc
import concourse.mybir as mybir
import concourse.tile as tile
from concourse.bass_utils import run_bass_kernel_spmd

N_CORES = 8
N = 2048
D = 256
H = 8
DK = 32
QS = 256
H2 = 512
EPS = 1e-5
NCH = 16  # key chunks of 128

F32 = mybir.dt.float32
BF = mybir.dt.bfloat16
F8 = mybir.dt.float8e4

AF = mybir.ActivationFunctionType
ALU = mybir.AluOpType
AX = mybir.AxisListType

# Head processing order: pairs sharing a PE quadrant row offset (bp) share
# a psum zero region.
ORDER = [0, 4, 1, 5, 2, 6, 3, 7]


def build_kernel():
    nc = bacc.Bacc("TRN2", target_bir_lowering=False, debug=False,
                   num_devices=N_CORES)

    d_hT = nc.dram_tensor("hT", [D, N], BF, kind="ExternalInput")
    d_hTs = nc.dram_tensor("hTs", [D, QS], BF, kind="ExternalInput")
    d_hres = nc.dram_tensor("hres", [QS, D], BF, kind="ExternalInput")
    d_wq = nc.dram_tensor("wq", [D, D], BF, kind="ExternalInput")
    d_wk = nc.dram_tensor("wk", [D, D], BF, kind="ExternalInput")
    d_wv = nc.dram_tensor("wv", [D, D], BF, kind="ExternalInput")
    d_wo = nc.dram_tensor("wo", [D, D], BF, kind="ExternalInput")
    d_w1 = nc.dram_tensor("w1", [D, H2], BF, kind="ExternalInput")
    d_w2 = nc.dram_tensor("w2", [H2, D], BF, kind="ExternalInput")
    d_cols = nc.dram_tensor("cols", [128, 12], F32, kind="ExternalInput")
    d_bo = nc.dram_tensor("bo_eff", [1, D], BF, kind="ExternalInput")
    d_lng = nc.dram_tensor("lng", [128, 4 * D], BF, kind="ExternalInput")
    d_bias = nc.dram_tensor("bias8", [NCH, 128, H, QS], F8,
                            kind="ExternalInput")
    d_id8 = nc.dram_tensor("ident8", [128, 128], F8, kind="ExternalInput")
    d_idb = nc.dram_tensor("identb", [128, 128], BF, kind="ExternalInput")
    d_out = nc.dram_tensor("out", [QS, D], F32, kind="ExternalOutput")

    with tile.TileContext(nc) as tc:
        import contextlib

        with contextlib.ExitStack() as ctx:
            wp = ctx.enter_context(tc.tile_pool(name="w", bufs=1))
            bpool = ctx.enter_context(tc.tile_pool(name="bias", bufs=5))
            ptp = ctx.enter_context(tc.tile_pool(name="pt", bufs=5))
            sm = ctx.enter_context(tc.tile_pool(name="sm", bufs=2))
            ps_sc = ctx.enter_context(
                tc.tile_pool(name="psc", bufs=2, space="PSUM"))
            ps_at = ctx.enter_context(
                tc.tile_pool(name="pat", bufs=1, space="PSUM"))
            ps_ms = ctx.enter_context(
                tc.tile_pool(name="pms", bufs=2, space="PSUM"))

            def load(pool, dram, shape, name, dt, r0=0, c0=0):
                t = pool.tile(shape, dt, name=name, tag=name)
                nc.sync.dma_start(
                    t[:], dram.ap()[r0:r0 + shape[0], c0:c0 + shape[1]])
                return t

            # ---- critical-path loads first ----
            hTs = [load(wp, d_hTs, [128, QS], f"hTs{i}", BF, 128 * i)
                   for i in range(2)]
            wq = [load(wp, d_wq, [128, D], f"wq{i}", BF, 128 * i)
                  for i in range(2)]
            cols = load(wp, d_cols, [128, 12], "cols", F32)
            # hT loaded in column halves so K-proj fc0/1 can start early
            hT = []
            for i in range(2):
                t = wp.tile([128, N], BF, name=f"hT{i}", tag=f"hT{i}")
                for half in range(2):
                    nc.sync.dma_start(
                        t[:, 1024 * half:1024 * half + 1024],
                        d_hT.ap()[128 * i:128 * i + 128,
                                  1024 * half:1024 * half + 1024])
                hT.append(t)
            wk = [load(wp, d_wk, [128, D], f"wk{i}", BF, 128 * i)
                  for i in range(2)]
            id8 = load(wp, d_id8, [128, 128], "id8", F8)
            wv = [load(wp, d_wv, [128, D], f"wv{i}", BF, 128 * i)
                  for i in range(2)]

            bq = [cols[:, 0:1], cols[:, 1:2]]
            bk = [cols[:, 2:3], cols[:, 3:4]]
            b1c = [cols[:, 4 + i:5 + i] for i in range(4)]
            b2c = [cols[:, 8:9], cols[:, 9:10]]
            zcol = cols[:, 10:11]
            epscol = cols[:, 11:12]

            ones = wp.tile([1, 128], BF, name="ones", tag="ones")
            nc.vector.memset(ones[:], 1.0)

            # V with per-head all-ones denominator column (col 32 of 33)
            v_sb = wp.tile([128, NCH, H, 33], BF, name="v_sb", tag="v_sb")
            nc.vector.memset(v_sb[:, :, :, 32:33], 1.0)

            bias_tiles = {}

            def emit_bias_dma(c):
                t = bpool.tile([128, H, QS], F8, tag="bias", name=f"bias{c}")
                nc.sync.dma_start(t[:], d_bias.ap()[c])
                bias_tiles[c] = t

            emit_bias_dma(0)
            emit_bias_dma(1)
            emit_bias_dma(2)

            # ---------- Q projection ----------
            QT = []
            for oc in range(2):
                ps = ps_ms.tile([128, 512], F32, tag="pms", name="psq")
                for ic in range(2):
                    nc.tensor.matmul(
                        ps[:, 0:QS], wq[ic][:, 128 * oc:128 * oc + 128],
                        hTs[ic][:], start=(ic == 0), stop=(ic == 1))
                t = wp.tile([128, QS], BF, name=f"QT{oc}", tag=f"QT{oc}")
                nc.vector.tensor_scalar_add(t[:], ps[:, 0:QS], bq[oc])
                QT.append(t)

            KT = [wp.tile([128, N], BF, name=f"KT{i}", tag=f"KT{i}")
                  for i in range(2)]
            att = [ps_at.tile([128, H, 33], F32, tag=f"att{qt}",
                              name=f"att{qt}") for qt in range(2)]
            for qt in range(2):
                nc.vector.memset(att[qt][:], 0.0)
            pt_tiles = {}

            def emit_kproj(fc):
                for oc in range(2):
                    ps = ps_ms.tile([128, 512], F32, tag="pms", name="psk")
                    for ic in range(2):
                        nc.tensor.matmul(
                            ps[:], wk[ic][:, 128 * oc:128 * oc + 128],
                            hT[ic][:, 512 * fc:512 * fc + 512],
                            start=(ic == 0), stop=(ic == 1))
                    nc.vector.tensor_scalar_add(
                        KT[oc][:, 512 * fc:512 * fc + 512], ps[:], bk[oc])

            def emit_vproj(c):
                ps = ps_ms.tile([128, 512], F32, tag="pms", name="psv")
                for ic in range(2):
                    nc.tensor.matmul(
                        ps[:, 0:D], hT[ic][:, 128 * c:128 * c + 128],
                        wv[ic][:], start=(ic == 0), stop=(ic == 1))
                nc.vector.tensor_copy(
                    v_sb[:, c, :, 0:32],
                    ps[:, 0:D].rearrange("p (h e) -> p h e", h=H))

            def emit_chunk(c):
                pt = ptp.tile([128, 2, 4, QS], BF, tag="pt", name=f"pt{c}")
                pt_tiles[c] = pt
                bt = bias_tiles.pop(c)
                for g in range(2):
                    ps = ps_sc.tile([128, 4, QS], F32, tag="sc",
                                    name=f"sc{c}_{g}")
                    for s in range(2):
                        nc.tensor.matmul(
                            ps[:, 2 * s:2 * s + 2, :], id8[:],
                            bt[:, 4 * g + 2 * s:4 * g + 2 * s + 2, :],
                            start=True, stop=False, skip_group_check=True)
                    for hh in range(4):
                        h = ORDER[4 * g + hh]
                        bp = 32 * (h % 4)
                        nc.tensor.matmul(
                            ps[:, hh, :],
                            KT[h // 4][bp:bp + 32, 128 * c:128 * c + 128],
                            QT[h // 4][bp:bp + 32, :],
                            start=False, stop=True, tile_position=(bp, 0),
                            skip_group_check=True)
                    nc.scalar.activation(pt[:, g], ps[:], AF.Exp, bias=zcol)

            def emit_attnv(c):
                pt = pt_tiles.pop(c)
                for qt in range(2):
                    for g in range(2):
                        for hh in range(4):
                            h = ORDER[4 * g + hh]
                            nc.tensor.matmul(
                                att[qt][:, h, :],
                                pt[:, g, hh, 128 * qt:128 * qt + 128],
                                v_sb[:, c, h, :],
                                start=False, stop=(c == NCH - 1),
                                skip_group_check=True)

            def emit_c(c):
                if c + 3 < NCH:
                    emit_bias_dma(c + 3)
                emit_vproj(c)
                emit_chunk(c)
                if c >= 2:
                    emit_attnv(c - 2)

            # ---------- main pipeline ----------
            emit_kproj(0)
            emit_c(0)
            emit_c(1)
            emit_kproj(1)
            emit_c(2)
            emit_c(3)
            emit_kproj(2)
            emit_c(4)
            emit_c(5)
            emit_kproj(3)

            # deferred (epilogue-only) loads
            hres = [load(wp, d_hres, [128, D], f"hres{i}", BF, 128 * i)
                    for i in range(2)]
            wo = [load(wp, d_wo, [128, D], f"wo{i}", BF, 128 * i)
                  for i in range(2)]
            w1 = [load(wp, d_w1, [128, H2], f"w1{i}", BF, 128 * i)
                  for i in range(2)]
            w2 = [load(wp, d_w2, [128, D], f"w2{i}", BF, 128 * i)
                  for i in range(4)]
            bo = load(wp, d_bo, [1, D], "bo", BF)
            lng = load(wp, d_lng, [128, 4 * D], "lng", BF)
            idb = load(wp, d_idb, [128, 128], "idb", BF)

            for c in range(6, NCH):
                emit_c(c)
            emit_attnv(NCH - 2)
            emit_attnv(NCH - 1)

            # ---------- epilogue (stage-interleaved across qt) ----------
            o_nat = [wp.tile([128, D], BF, name=f"onat{qt}", tag=f"onat{qt}")
                     for qt in range(2)]
            OT = [wp.tile([128, D], BF, name=f"OT{fc}", tag=f"OT{fc}")
                  for fc in range(2)]
            fT = [wp.tile([128, D], BF, name=f"fT{fc}", tag=f"fT{fc}")
                  for fc in range(2)]
            g1T = [wp.tile([128, QS], BF, name=f"g1T{oc}", tag=f"g1T{oc}")
                   for oc in range(4)]
            y2T = [wp.tile([128, QS], BF, name=f"y2T{oc}", tag=f"y2T{oc}")
                   for oc in range(2)]
            h1 = [wp.tile([128, D], F32, name=f"h1_{qt}", tag=f"h1_{qt}")
                  for qt in range(2)]
            xin = [sm.tile([128, D], F32, tag=f"xin{qt}", name=f"xin{qt}")
                   for qt in range(2)]
            fln = [sm.tile([128, D], BF, tag=f"fln{qt}", name=f"fln{qt}")
                   for qt in range(2)]
            out_sb = [wp.tile([128, D], F32, name=f"osb{qt}", tag=f"osb{qt}")
                      for qt in range(2)]

            # normalize attention output
            for qt in range(2):
                rdt = sm.tile([128, H, 1], F32, tag=f"rd{qt}",
                              name=f"rd{qt}")
                nc.vector.reciprocal(rdt[:], att[qt][:, :, 32:33])
                for h in range(8):
                    nc.vector.tensor_scalar_mul(
                        o_nat[qt][:, 32 * h:32 * h + 32],
                        att[qt][:, h, 0:32], rdt[:, h])
            # transpose o_nat -> OT
            for qt in range(2):
                for fc in range(2):
                    tp = ps_ms.tile([128, 512], BF, tag="pms", name="trp")
                    nc.tensor.transpose(
                        tp[:, 0:128], o_nat[qt][:, 128 * fc:128 * fc + 128],
                        idb[:])
                    nc.vector.tensor_copy(
                        OT[fc][:, 128 * qt:128 * qt + 128], tp[:, 0:128])
            # out-projection + residual
            for qt in range(2):
                aps = ps_ms.tile([128, 512], F32, tag="pms", name="apo")
                for ic in range(2):
                    nc.tensor.matmul(
                        aps[:, 0:D], OT[ic][:, 128 * qt:128 * qt + 128],
                        wo[ic][:], start=(ic == 0), stop=False)
                nc.tensor.matmul(aps[:, 0:D], ones[:], bo[:],
                                 start=False, stop=True)
                nc.vector.tensor_add(xin[qt][:], aps[:, 0:D], hres[qt][:])

            def ln_stats(x_ap, tagp):
                """Emit LN stats for one tile; returns (xc, st) where
                rstd is computed later (reciprocal of st)."""
                ss = sm.tile([128, 1], F32, tag=f"{tagp}ss", name="ss")
                nc.vector.reduce_sum(ss[:], x_ap, axis=AX.X)
                nm = sm.tile([128, 1], F32, tag=f"{tagp}nm", name="nm")
                nc.vector.tensor_scalar_mul(nm[:], ss[:], -1.0 / D)
                xc = sm.tile([128, D], F32, tag=f"{tagp}xc", name="xc")
                nc.vector.tensor_scalar_add(xc[:], x_ap, nm[:])
                scr = sm.tile([128, D], F32, tag=f"{tagp}scr", name="scr")
                nc.vector.tensor_mul(scr[:], xc[:], xc[:])
                vs = sm.tile([128, 1], F32, tag=f"{tagp}vs", name="vs")
                nc.vector.reduce_sum(vs[:], scr[:], axis=AX.X)
                st = sm.tile([128, 1], F32, tag=f"{tagp}st", name="st")
                nc.scalar.activation(st[:], vs[:], AF.Sqrt, bias=epscol,
                                     scale=1.0 / D)
                return xc, st

            def ln_apply(xc, st, g_ap, b_ap, out_ap, tagp):
                r0 = sm.tile([128, 1], F32, tag=f"{tagp}r0", name="r0")
                nc.vector.reciprocal(r0[:], st[:])
                yp = sm.tile([128, D], F32, tag=f"{tagp}yp", name="yp")
                nc.vector.tensor_scalar_mul(yp[:], xc[:], r0[:])
                yg = sm.tile([128, D], F32, tag=f"{tagp}yg", name="yg")
                nc.vector.tensor_mul(yg[:], yp[:], g_ap)
                nc.vector.tensor_add(out_ap, yg[:], b_ap)

            # LN1 both qt (Sqrt ops batched)
            st1 = []
            for qt in range(2):
                st1.append(ln_stats(xin[qt][:], f"l1q{qt}"))
            for qt in range(2):
                xc, st = st1[qt]
                ln_apply(xc, st, lng[:, 0:D], lng[:, D:2 * D], h1[qt][:],
                         f"l1q{qt}")
            # LN2 both qt
            st2 = []
            for qt in range(2):
                st2.append(ln_stats(h1[qt][:], f"l2q{qt}"))
            for qt in range(2):
                xc, st = st2[qt]
                ln_apply(xc, st, lng[:, 2 * D:3 * D], lng[:, 3 * D:4 * D],
                         fln[qt][:], f"l2q{qt}")
            # transpose fln -> fT
            for qt in range(2):
                for fc in range(2):
                    tp2 = ps_ms.tile([128, 512], BF, tag="pms", name="trf")
                    nc.tensor.transpose(
                        tp2[:, 0:128], fln[qt][:, 128 * fc:128 * fc + 128],
                        idb[:])
                    nc.vector.tensor_copy(
                        fT[fc][:, 128 * qt:128 * qt + 128], tp2[:, 0:128])
            # FFN1 + gelu (gelus batched)
            for qt in range(2):
                for oc in range(4):
                    fp = ps_ms.tile([128, 512], F32, tag="pms", name="ps1")
                    for ic in range(2):
                        nc.tensor.matmul(
                            fp[:, 0:128],
                            w1[ic][:, 128 * oc:128 * oc + 128],
                            fT[ic][:, 128 * qt:128 * qt + 128],
                            start=(ic == 0), stop=(ic == 1))
                    nc.scalar.activation(
                        g1T[oc][:, 128 * qt:128 * qt + 128], fp[:, 0:128],
                        AF.Gelu, bias=b1c[oc])
            # FFN2
            for qt in range(2):
                for oc in range(2):
                    fp = ps_ms.tile([128, 512], F32, tag="pms", name="ps2")
                    for ic in range(4):
                        nc.tensor.matmul(
                            fp[:, 0:128],
                            w2[ic][:, 128 * oc:128 * oc + 128],
                            g1T[ic][:, 128 * qt:128 * qt + 128],
                            start=(ic == 0), stop=(ic == 3))
                    nc.vector.tensor_scalar_add(
                        y2T[oc][:, 128 * qt:128 * qt + 128], fp[:, 0:128],
                        b2c[oc])
            # transpose back + residual + store
            for qt in range(2):
                for fc in range(2):
                    tp3 = ps_ms.tile([128, 512], BF, tag="pms", name="trn")
                    nc.tensor.transpose(
                        tp3[:, 0:128], y2T[fc][:, 128 * qt:128 * qt + 128],
                        idb[:])
                    nc.vector.tensor_add(
                        out_sb[qt][:, 128 * fc:128 * fc + 128],
                        h1[qt][:, 128 * fc:128 * fc + 128], tp3[:, 0:128])
                nc.sync.dma_start(d_out.ap()[128 * qt:128 * qt + 128, :],
                                  out_sb[qt][:])

    nc.compile()
    return nc


_CACHE = {}
USE_FR = True


def _get_nc(use_fr=True):
    if use_fr not in _CACHE:
        _CACHE[use_fr] = build_kernel()
    return _CACHE[use_fr]


def kernel(**inputs):
    import ml_dtypes

    bf = ml_dtypes.bfloat16
    f8 = ml_dtypes.float8_e4m3fn

    h = np.asarray(inputs["h"], np.float32)
    edge_attr = np.asarray(inputs["edge_attr"], np.float32)
    edge_index = np.asarray(inputs["edge_index"])
    Wq, bq = np.asarray(inputs["Wq"], np.float32), np.asarray(inputs["bq"], np.float32)
    Wk, bk = np.asarray(inputs["Wk"], np.float32), np.asarray(inputs["bk"], np.float32)
    Wv, bv = np.asarray(inputs["Wv"], np.float32), np.asarray(inputs["bv"], np.float32)
    Wo, bo = np.asarray(inputs["Wo"], np.float32), np.asarray(inputs["bo"], np.float32)
    We, be = np.asarray(inputs["We"], np.float32), np.asarray(inputs["be"], np.float32)
    ln1_g, ln1_b = np.asarray(inputs["ln1_g"], np.float32), np.asarray(inputs["ln1_b"], np.float32)
    fln_g, fln_b = np.asarray(inputs["fln_g"], np.float32), np.asarray(inputs["fln_b"], np.float32)
    W1, b1 = np.asarray(inputs["W1"], np.float32), np.asarray(inputs["b1"], np.float32)
    W2, b2 = np.asarray(inputs["W2"], np.float32), np.asarray(inputs["b2"], np.float32)

    scale = 1.0 / np.sqrt(np.float32(DK))
    eb = edge_attr @ We + be  # (E, H)
    hT = np.ascontiguousarray(h.T)

    cols = np.zeros((128, 12), np.float32)
    cols[:, 0] = bq[0:128] * scale
    cols[:, 1] = bq[128:256] * scale
    cols[:, 2] = bk[0:128]
    cols[:, 3] = bk[128:256]
    for i in range(4):
        cols[:, 4 + i] = b1[128 * i:128 * i + 128]
    cols[:, 8] = b2[0:128]
    cols[:, 9] = b2[128:256]
    cols[:, 11] = EPS

    bo_eff = (bv @ Wo + bo).reshape(1, D)
    lng = np.concatenate(
        [np.tile(x.reshape(1, D), (128, 1))
         for x in [ln1_g, ln1_b, fln_g, fln_b]], axis=1)

    common = {
        "hT": hT.astype(bf),
        "wq": (Wq * scale).astype(bf),
        "wk": Wk.astype(bf),
        "wv": Wv.astype(bf),
        "wo": Wo.astype(bf),
        "w1": W1.astype(bf),
        "w2": W2.astype(bf),
        "cols": cols,
        "bo_eff": bo_eff.astype(bf),
        "lng": lng.astype(bf),
        "ident8": np.eye(128, dtype=np.float32).astype(f8),
        "identb": np.eye(128, dtype=np.float32).astype(bf),
    }

    src = edge_index[0].astype(np.int64)
    dst = edge_index[1].astype(np.int64)
    in_maps = []
    for c in range(N_CORES):
        r0 = c * QS
        m = dict(common)
        m["hTs"] = np.ascontiguousarray(hT[:, r0:r0 + QS]).astype(bf)
        m["hres"] = np.ascontiguousarray(h[r0:r0 + QS]).astype(bf)
        sel = (src >= r0) & (src < r0 + QS)
        slab = np.zeros((N, H, QS), np.float32)
        slab[dst[sel], :, src[sel] - r0] = eb[sel]
        slab = slab[:, ORDER, :]  # head slot order
        m["bias8"] = np.ascontiguousarray(
            slab.reshape(NCH, 128, H, QS)).astype(f8)
        in_maps.append(m)

    nc = _get_nc(USE_FR)
    res = run_bass_kernel_spmd(nc, in_maps, core_ids=list(range(N_CORES)))
    out = np.concatenate([res.results[cc]["out"] for cc in range(N_CORES)],
                         axis=0)
    return out.astype(np.float32)


# revision 17
# speedup vs baseline: 2.5073x; 1.0151x over previous
"""GraphTransformerLayer on 8 TRN2 NeuronCores (Bass/Tile).

Sharding: query/node dim N=2048 split into 8 shards of 256 rows; each core
holds replicated K/V for all 2048 keys plus its 256-query shard.

Design (v3):
- All matmuls bf16 (fp32 runs at 1/4 PE rate); psum accumulation f32.
- Edge bias: dense per-core fp8 slab [16][128 keys, 8 heads, 256 queries],
  added into the score PSUM by seeding each accumulation group with an
  fp8 identity-matmul (start=True) before the K^T Q score matmuls
  (start=False, stop=True) accumulate on top.
- Heads are processed in the order [0,4,1,5,2,6,3,7] so that the two heads
  sharing one 2KB psum zero-region use the same PE quadrant row offset —
  mixing different sub-128 partition offsets in one zero region hangs the
  PE on hardware.
- Scores chunk-major: per key-chunk c (128 keys) and half g (4 heads), one
  [128, 4x256] psum tile -> one Exp activation psum->SBUF bf16 (pt).
  Unnormalized softmax; denominator comes from an all-ones column per head
  appended to V (attn @ [V|1]).
- attnV accumulates into memset-seeded persistent [128, 8, 33] psum tiles
  per query half (start=False throughout).
- V projection is spread across the chunk loop as PE filler; epilogue is
  stage-interleaved across the two query halves with activation-table
  switches batched (Exp -> Sqrt -> Gelu).
"""

import sys

sys.path.insert(0, "/opt/trn_rl_repo")

import numpy as np

import concourse.bacc as bacc
import concourse.mybir as mybir
import concourse.tile as tile
from concourse.bass_utils import run_bass_kernel_spmd

N_CORES = 8
N = 2048
D = 256
H = 8
DK = 32
QS = 256
H2 = 512
EPS = 1e-5
NCH = 16  # key chunks of 128

F32 = mybir.dt.float32
BF = mybir.dt.bfloat16
F8 = mybir.dt.float8e4

AF = mybir.ActivationFunctionType
ALU = mybir.AluOpType
AX = mybir.AxisListType

# Head processing order: pairs sharing a PE quadrant row offset (bp) share
# a psum zero region.
ORDER = [0, 4, 1, 5, 2, 6, 3, 7]


def build_kernel():
    nc = bacc.Bacc("TRN2", target_bir_lowering=False, debug=False,
                   num_devices=N_CORES)

    d_hT = nc.dram_tensor("hT", [D, N], BF, kind="ExternalInput")
    d_hTs = nc.dram_tensor("hTs", [D, QS], BF, kind="ExternalInput")
    d_hres = nc.dram_tensor("hres", [QS, D], BF, kind="ExternalInput")
    d_wq = nc.dram_tensor("wq", [D, D], BF, kind="ExternalInput")
    d_wk = nc.dram_tensor("wk", [D, D], BF, kind="ExternalInput")
    d_wv = nc.dram_tensor("wv", [D, D], BF, kind="ExternalInput")
    d_wo = nc.dram_tensor("wo", [D, D], BF, kind="ExternalInput")
    d_w1 = nc.dram_tensor("w1", [D, H2], BF, kind="ExternalInput")
    d_w2 = nc.dram_tensor("w2", [H2, D], BF, kind="ExternalInput")
    d_cols = nc.dram_tensor("cols", [128, 12], F32, kind="ExternalInput")
    d_bo = nc.dram_tensor("bo_eff", [1, D], BF, kind="ExternalInput")
    d_lng = nc.dram_tensor("lng", [128, 4 * D], BF, kind="ExternalInput")
    d_bias = nc.dram_tensor("bias8", [NCH, 128, H, QS], F8,
                            kind="ExternalInput")
    d_id8 = nc.dram_tensor("ident8", [128, 128], F8, kind="ExternalInput")
    d_idb = nc.dram_tensor("identb", [128, 128], BF, kind="ExternalInput")
    d_out = nc.dram_tensor("out", [QS, D], F32, kind="ExternalOutput")

    with tile.TileContext(nc) as tc:
        import contextlib

        with contextlib.ExitStack() as ctx:
            wp = ctx.enter_context(tc.tile_pool(name="w", bufs=1))
            bpool = ctx.enter_context(tc.tile_pool(name="bias", bufs=5))
            ptp = ctx.enter_context(tc.tile_pool(name="pt", bufs=5))
            sm = ctx.enter_context(tc.tile_pool(name="sm", bufs=2))
            ps_sc = ctx.enter_context(
                tc.tile_pool(name="psc", bufs=2, space="PSUM"))
            ps_at = ctx.enter_context(
                tc.tile_pool(name="pat", bufs=1, space="PSUM"))
            ps_ms = ctx.enter_context(
                tc.tile_pool(name="pms", bufs=2, space="PSUM"))

            def load(pool, dram, shape, name, dt, r0=0, c0=0):
                t = pool.tile(shape, dt, name=name, tag=name)
                nc.sync.dma_start(
                    t[:], dram.ap()[r0:r0 + shape[0], c0:c0 + shape[1]])
                return t

            # ---- critical-path loads first ----
            hTs = [load(wp, d_hTs, [128, QS], f"hTs{i}", BF, 128 * i)
                   for i in range(2)]
            wq = [load(wp, d_wq, [128, D], f"wq{i}", BF, 128 * i)
                  for i in range(2)]
            cols = load(wp, d_cols, [128, 12], "cols", F32)
            id8 = load(wp, d_id8, [128, 128], "id8", F8)

            bias_tiles = {}

            def emit_bias_dma(c):
                t = bpool.tile([128, H, QS], F8, tag="bias", name=f"bias{c}")
                nc.sync.dma_start(t[:], d_bias.ap()[c])
                bias_tiles[c] = t

            emit_bias_dma(0)
            # hT loaded in column halves so K-proj fc0/1 can start early
            hT = []
            for i in range(2):
                t = wp.tile([128, N], BF, name=f"hT{i}", tag=f"hT{i}")
                for half in range(2):
                    nc.sync.dma_start(
                        t[:, 1024 * half:1024 * half + 1024],
                        d_hT.ap()[128 * i:128 * i + 128,
                                  1024 * half:1024 * half + 1024])
                hT.append(t)
            wk = [load(wp, d_wk, [128, D], f"wk{i}", BF, 128 * i)
                  for i in range(2)]
            emit_bias_dma(1)
            wv = [load(wp, d_wv, [128, D], f"wv{i}", BF, 128 * i)
                  for i in range(2)]
            emit_bias_dma(2)

            bq = [cols[:, 0:1], cols[:, 1:2]]
            bk = [cols[:, 2:3], cols[:, 3:4]]
            b1c = [cols[:, 4 + i:5 + i] for i in range(4)]
            b2c = [cols[:, 8:9], cols[:, 9:10]]
            zcol = cols[:, 10:11]
            epscol = cols[:, 11:12]

            ones = wp.tile([1, 128], BF, name="ones", tag="ones")
            nc.vector.memset(ones[:], 1.0)

            # V with per-head all-ones denominator column (col 32 of 33)
            v_sb = wp.tile([128, NCH, H, 33], BF, name="v_sb", tag="v_sb")
            nc.vector.memset(v_sb[:, :, :, 32:33], 1.0)

            # ---------- Q projection ----------
            QT = []
            for oc in range(2):
                ps = ps_ms.tile([128, 512], F32, tag="pms", name="psq")
                for ic in range(2):
                    nc.tensor.matmul(
                        ps[:, 0:QS], wq[ic][:, 128 * oc:128 * oc + 128],
                        hTs[ic][:], start=(ic == 0), stop=(ic == 1))
                t = wp.tile([128, QS], BF, name=f"QT{oc}", tag=f"QT{oc}")
                nc.vector.tensor_scalar_add(t[:], ps[:, 0:QS], bq[oc])
                QT.append(t)

            KT = [wp.tile([128, N], BF, name=f"KT{i}", tag=f"KT{i}")
                  for i in range(2)]
            att = [ps_at.tile([128, H, 33], F32, tag=f"att{qt}",
                              name=f"att{qt}") for qt in range(2)]
            for qt in range(2):
                nc.vector.memset(att[qt][:], 0.0)
            pt_tiles = {}

            def emit_kproj(fc):
                for oc in range(2):
                    ps = ps_ms.tile([128, 512], F32, tag="pms", name="psk")
                    for ic in range(2):
                        nc.tensor.matmul(
                            ps[:], wk[ic][:, 128 * oc:128 * oc + 128],
                            hT[ic][:, 512 * fc:512 * fc + 512],
                            start=(ic == 0), stop=(ic == 1))
                    nc.vector.tensor_scalar_add(
                        KT[oc][:, 512 * fc:512 * fc + 512], ps[:], bk[oc])

            def emit_vproj(c):
                ps = ps_ms.tile([128, 512], F32, tag="pms", name="psv")
                for ic in range(2):
                    nc.tensor.matmul(
                        ps[:, 0:D], hT[ic][:, 128 * c:128 * c + 128],
                        wv[ic][:], start=(ic == 0), stop=(ic == 1))
                nc.vector.tensor_copy(
                    v_sb[:, c, :, 0:32],
                    ps[:, 0:D].rearrange("p (h e) -> p h e", h=H))

            def emit_chunk(c):
                pt = ptp.tile([128, 2, 4, QS], BF, tag="pt", name=f"pt{c}")
                pt_tiles[c] = pt
                bt = bias_tiles.pop(c)
                for g in range(2):
                    ps = ps_sc.tile([128, 4, QS], F32, tag="sc",
                                    name=f"sc{c}_{g}")
                    for s in range(2):
                        nc.tensor.matmul(
                            ps[:, 2 * s:2 * s + 2, :], id8[:],
                            bt[:, 4 * g + 2 * s:4 * g + 2 * s + 2, :],
                            start=True, stop=False, skip_group_check=True)
                    for hh in range(4):
                        h = ORDER[4 * g + hh]
                        bp = 32 * (h % 4)
                        nc.tensor.matmul(
                            ps[:, hh, :],
                            KT[h // 4][bp:bp + 32, 128 * c:128 * c + 128],
                            QT[h // 4][bp:bp + 32, :],
                            start=False, stop=True, tile_position=(bp, 0),
                            skip_group_check=True)
                    nc.scalar.activation(pt[:, g], ps[:], AF.Exp, bias=zcol)

            def emit_attnv(c):
                pt = pt_tiles.pop(c)
                for qt in range(2):
                    for g in range(2):
                        for hh in range(4):
                            h = ORDER[4 * g + hh]
                            nc.tensor.matmul(
                                att[qt][:, h, :],
                                pt[:, g, hh, 128 * qt:128 * qt + 128],
                                v_sb[:, c, h, :],
                                start=False, stop=(c == NCH - 1),
                                skip_group_check=True)

            def emit_c(c):
                if c + 3 < NCH:
                    emit_bias_dma(c + 3)
                emit_vproj(c)
                emit_chunk(c)
                if c >= 2:
                    emit_attnv(c - 2)

            # ---------- main pipeline ----------
            emit_kproj(0)
            emit_c(0)
            emit_c(1)
            emit_kproj(1)
            emit_c(2)
            emit_c(3)
            emit_kproj(2)
            emit_c(4)
            emit_c(5)
            emit_kproj(3)

            # deferred (epilogue-only) loads
            hres = [load(wp, d_hres, [128, D], f"hres{i}", BF, 128 * i)
                    for i in range(2)]
            wo = [load(wp, d_wo, [128, D], f"wo{i}", BF, 128 * i)
                  for i in range(2)]
            w1 = [load(wp, d_w1, [128, H2], f"w1{i}", BF, 128 * i)
                  for i in range(2)]
            w2 = [load(wp, d_w2, [128, D], f"w2{i}", BF, 128 * i)
                  for i in range(4)]
            bo = load(wp, d_bo, [1, D], "bo", BF)
            lng = load(wp, d_lng, [128, 4 * D], "lng", BF)
            idb = load(wp, d_idb, [128, 128], "idb", BF)

            for c in range(6, NCH):
                emit_c(c)
            emit_attnv(NCH - 2)
            emit_attnv(NCH - 1)

            # ---------- epilogue (stage-interleaved across qt) ----------
            o_nat = [wp.tile([128, D], BF, name=f"onat{qt}", tag=f"onat{qt}")
                     for qt in range(2)]
            OT = [wp.tile([128, D], BF, name=f"OT{fc}", tag=f"OT{fc}")
                  for fc in range(2)]
            fT = [wp.tile([128, D], BF, name=f"fT{fc}", tag=f"fT{fc}")
                  for fc in range(2)]
            g1T = [wp.tile([128, QS], BF, name=f"g1T{oc}", tag=f"g1T{oc}")
                   for oc in range(4)]
            y2T = [wp.tile([128, QS], BF, name=f"y2T{oc}", tag=f"y2T{oc}")
                   for oc in range(2)]
            h1 = [wp.tile([128, D], F32, name=f"h1_{qt}", tag=f"h1_{qt}")
                  for qt in range(2)]
            xin = [sm.tile([128, D], F32, tag=f"xin{qt}", name=f"xin{qt}")
                   for qt in range(2)]
            fln = [sm.tile([128, D], BF, tag=f"fln{qt}", name=f"fln{qt}")
                   for qt in range(2)]
            out_sb = [wp.tile([128, D], F32, name=f"osb{qt}", tag=f"osb{qt}")
                      for qt in range(2)]

            # normalize attention output
            for qt in range(2):
                rdt = sm.tile([128, H, 1], F32, tag=f"rd{qt}",
                              name=f"rd{qt}")
                nc.vector.reciprocal(rdt[:], att[qt][:, :, 32:33])
                for h in range(8):
                    nc.vector.tensor_scalar_mul(
                        o_nat[qt][:, 32 * h:32 * h + 32],
                        att[qt][:, h, 0:32], rdt[:, h])
            # transpose o_nat -> OT
            for qt in range(2):
                for fc in range(2):
                    tp = ps_ms.tile([128, 512], BF, tag="pms", name="trp")
                    nc.tensor.transpose(
                        tp[:, 0:128], o_nat[qt][:, 128 * fc:128 * fc + 128],
                        idb[:])
                    nc.vector.tensor_copy(
                        OT[fc][:, 128 * qt:128 * qt + 128], tp[:, 0:128])
            # out-projection + residual
            for qt in range(2):
                aps = ps_ms.tile([128, 512], F32, tag="pms", name="apo")
                for ic in range(2):
                    nc.tensor.matmul(
                        aps[:, 0:D], OT[ic][:, 128 * qt:128 * qt + 128],
                        wo[ic][:], start=(ic == 0), stop=False)
                nc.tensor.matmul(aps[:, 0:D], ones[:], bo[:],
                                 start=False, stop=True)
                nc.vector.tensor_add(xin[qt][:], aps[:, 0:D], hres[qt][:])

            def ln_stats(x_ap, tagp, eng):
                """Emit LN stats for one tile; returns (xc, st) where
                rstd is computed later (reciprocal of st)."""
                ss = sm.tile([128, 1], F32, tag=f"{tagp}ss", name="ss")
                nc.vector.reduce_sum(ss[:], x_ap, axis=AX.X)
                nm = sm.tile([128, 1], F32, tag=f"{tagp}nm", name="nm")
                eng.tensor_scalar_mul(nm[:], ss[:], -1.0 / D)
                xc = sm.tile([128, D], F32, tag=f"{tagp}xc", name="xc")
                eng.tensor_scalar_add(xc[:], x_ap, nm[:])
                scr = sm.tile([128, D], F32, tag=f"{tagp}scr", name="scr")
                eng.tensor_mul(scr[:], xc[:], xc[:])
                vs = sm.tile([128, 1], F32, tag=f"{tagp}vs", name="vs")
                nc.vector.reduce_sum(vs[:], scr[:], axis=AX.X)
                st = sm.tile([128, 1], F32, tag=f"{tagp}st", name="st")
                nc.scalar.activation(st[:], vs[:], AF.Sqrt, bias=epscol,
                                     scale=1.0 / D)
                return xc, st

            def ln_apply(xc, st, g_ap, b_ap, out_ap, tagp, eng):
                r0 = sm.tile([128, 1], F32, tag=f"{tagp}r0", name="r0")
                nc.vector.reciprocal(r0[:], st[:])
                yp = sm.tile([128, D], F32, tag=f"{tagp}yp", name="yp")
                eng.tensor_scalar_mul(yp[:], xc[:], r0[:])
                yg = sm.tile([128, D], F32, tag=f"{tagp}yg", name="yg")
                eng.tensor_mul(yg[:], yp[:], g_ap)
                eng.tensor_add(out_ap, yg[:], b_ap)

            LNE = [nc.vector, nc.gpsimd]

            # LN1 both qt (Sqrt ops batched; qt1 runs on gpsimd)
            st1 = []
            for qt in range(2):
                st1.append(ln_stats(xin[qt][:], f"l1q{qt}", LNE[qt]))
            for qt in range(2):
                xc, st = st1[qt]
                ln_apply(xc, st, lng[:, 0:D], lng[:, D:2 * D], h1[qt][:],
                         f"l1q{qt}", LNE[qt])
            # LN2 both qt
            st2 = []
            for qt in range(2):
                st2.append(ln_stats(h1[qt][:], f"l2q{qt}", LNE[qt]))
            for qt in range(2):
                xc, st = st2[qt]
                ln_apply(xc, st, lng[:, 2 * D:3 * D], lng[:, 3 * D:4 * D],
                         fln[qt][:], f"l2q{qt}", LNE[qt])
            # transpose fln -> fT
            for qt in range(2):
                for fc in range(2):
                    tp2 = ps_ms.tile([128, 512], BF, tag="pms", name="trf")
                    nc.tensor.transpose(
                        tp2[:, 0:128], fln[qt][:, 128 * fc:128 * fc + 128],
                        idb[:])
                    nc.vector.tensor_copy(
                        fT[fc][:, 128 * qt:128 * qt + 128], tp2[:, 0:128])
            # FFN1 + gelu (gelus batched)
            for qt in range(2):
                for oc in range(4):
                    fp = ps_ms.tile([128, 512], F32, tag="pms", name="ps1")
                    for ic in range(2):
                        nc.tensor.matmul(
                            fp[:, 0:128],
                            w1[ic][:, 128 * oc:128 * oc + 128],
                            fT[ic][:, 128 * qt:128 * qt + 128],
                            start=(ic == 0), stop=(ic == 1))
                    nc.scalar.activation(
                        g1T[oc][:, 128 * qt:128 * qt + 128], fp[:, 0:128],
                        AF.Gelu, bias=b1c[oc])
            # FFN2
            for qt in range(2):
                for oc in range(2):
                    fp = ps_ms.tile([128, 512], F32, tag="pms", name="ps2")
                    for ic in range(4):
                        nc.tensor.matmul(
                            fp[:, 0:128],
                            w2[ic][:, 128 * oc:128 * oc + 128],
                            g1T[ic][:, 128 * qt:128 * qt + 128],
                            start=(ic == 0), stop=(ic == 3))
                    nc.vector.tensor_scalar_add(
                        y2T[oc][:, 128 * qt:128 * qt + 128], fp[:, 0:128],
                        b2c[oc])
            # transpose back + residual + store
            for qt in range(2):
                for fc in range(2):
                    tp3 = ps_ms.tile([128, 512], BF, tag="pms", name="trn")
                    nc.tensor.transpose(
                        tp3[:, 0:128], y2T[fc][:, 128 * qt:128 * qt + 128],
                        idb[:])
                    nc.vector.tensor_add(
                        out_sb[qt][:, 128 * fc:128 * fc + 128],
                        h1[qt][:, 128 * fc:128 * fc + 128], tp3[:, 0:128])
                nc.sync.dma_start(d_out.ap()[128 * qt:128 * qt + 128, :],
                                  out_sb[qt][:])

    nc.compile()
    return nc


_CACHE = {}
USE_FR = True


def _get_nc(use_fr=True):
    if use_fr not in _CACHE:
        _CACHE[use_fr] = build_kernel()
    return _CACHE[use_fr]


def kernel(**inputs):
    import ml_dtypes

    bf = ml_dtypes.bfloat16
    f8 = ml_dtypes.float8_e4m3fn

    h = np.asarray(inputs["h"], np.float32)
    edge_attr = np.asarray(inputs["edge_attr"], np.float32)
    edge_index = np.asarray(inputs["edge_index"])
    Wq, bq = np.asarray(inputs["Wq"], np.float32), np.asarray(inputs["bq"], np.float32)
    Wk, bk = np.asarray(inputs["Wk"], np.float32), np.asarray(inputs["bk"], np.float32)
    Wv, bv = np.asarray(inputs["Wv"], np.float32), np.asarray(inputs["bv"], np.float32)
    Wo, bo = np.asarray(inputs["Wo"], np.float32), np.asarray(inputs["bo"], np.float32)
    We, be = np.asarray(inputs["We"], np.float32), np.asarray(inputs["be"], np.float32)
    ln1_g, ln1_b = np.asarray(inputs["ln1_g"], np.float32), np.asarray(inputs["ln1_b"], np.float32)
    fln_g, fln_b = np.asarray(inputs["fln_g"], np.float32), np.asarray(inputs["fln_b"], np.float32)
    W1, b1 = np.asarray(inputs["W1"], np.float32), np.asarray(inputs["b1"], np.float32)
    W2, b2 = np.asarray(inputs["W2"], np.float32), np.asarray(inputs["b2"], np.float32)

    scale = 1.0 / np.sqrt(np.float32(DK))
    eb = edge_attr @ We + be  # (E, H)
    hT = np.ascontiguousarray(h.T)

    cols = np.zeros((128, 12), np.float32)
    cols[:, 0] = bq[0:128] * scale
    cols[:, 1] = bq[128:256] * scale
    cols[:, 2] = bk[0:128]
    cols[:, 3] = bk[128:256]
    for i in range(4):
        cols[:, 4 + i] = b1[128 * i:128 * i + 128]
    cols[:, 8] = b2[0:128]
    cols[:, 9] = b2[128:256]
    cols[:, 11] = EPS

    bo_eff = (bv @ Wo + bo).reshape(1, D)
    lng = np.concatenate(
        [np.tile(x.reshape(1, D), (128, 1))
         for x in [ln1_g, ln1_b, fln_g, fln_b]], axis=1)

    common = {
        "hT": hT.astype(bf),
        "wq": (Wq * scale).astype(bf),
        "wk": Wk.astype(bf),
        "wv": Wv.astype(bf),
        "wo": Wo.astype(bf),
        "w1": W1.astype(bf),
        "w2": W2.astype(bf),
        "cols": cols,
        "bo_eff": bo_eff.astype(bf),
        "lng": lng.astype(bf),
        "ident8": np.eye(128, dtype=np.float32).astype(f8),
        "identb": np.eye(128, dtype=np.float32).astype(bf),
    }

    src = edge_index[0].astype(np.int64)
    dst = edge_index[1].astype(np.int64)
    in_maps = []
    for c in range(N_CORES):
        r0 = c * QS
        m = dict(common)
        m["hTs"] = np.ascontiguousarray(hT[:, r0:r0 + QS]).astype(bf)
        m["hres"] = np.ascontiguousarray(h[r0:r0 + QS]).astype(bf)
        sel = (src >= r0) & (src < r0 + QS)
        slab = np.zeros((N, H, QS), np.float32)
        slab[dst[sel], :, src[sel] - r0] = eb[sel]
        slab = slab[:, ORDER, :]  # head slot order
        m["bias8"] = np.ascontiguousarray(
            slab.reshape(NCH, 128, H, QS)).astype(f8)
        in_maps.append(m)

    nc = _get_nc(USE_FR)
    res = run_bass_kernel_spmd(nc, in_maps, core_ids=list(range(N_CORES)))
    out = np.concatenate([res.results[cc]["out"] for cc in range(N_CORES)],
                         axis=0)
    return out.astype(np.float32)


# revision 18
# speedup vs baseline: 2.8156x; 1.1230x over previous
"""GraphTransformerLayer on 8 TRN2 NeuronCores (Bass/Tile).

Sharding: query/node dim N=2048 split into 8 shards of 256 rows; each core
holds replicated K/V for all 2048 keys plus its 256-query shard.

Design (v3):
- All matmuls bf16 (fp32 runs at 1/4 PE rate); psum accumulation f32.
- Edge bias: dense per-core fp8 slab [16][128 keys, 8 heads, 256 queries],
  added into the score PSUM by seeding each accumulation group with an
  fp8 identity-matmul (start=True) before the K^T Q score matmuls
  (start=False, stop=True) accumulate on top.
- Heads are processed in the order [0,4,1,5,2,6,3,7] so that the two heads
  sharing one 2KB psum zero-region use the same PE quadrant row offset —
  mixing different sub-128 partition offsets in one zero region hangs the
  PE on hardware.
- Scores chunk-major: per key-chunk c (128 keys) and half g (4 heads), one
  [128, 4x256] psum tile -> one Exp activation psum->SBUF bf16 (pt).
  Unnormalized softmax; denominator comes from an all-ones column per head
  appended to V (attn @ [V|1]).
- attnV accumulates into memset-seeded persistent [128, 8, 33] psum tiles
  per query half (start=False throughout).
- V projection is spread across the chunk loop as PE filler; epilogue is
  stage-interleaved across the two query halves with activation-table
  switches batched (Exp -> Sqrt -> Gelu).
"""

import sys

sys.path.insert(0, "/opt/trn_rl_repo")

import numpy as np

import concourse.bacc as bacc
import concourse.mybir as mybir
import concourse.tile as tile
from concourse.bass_utils import run_bass_kernel_spmd

N_CORES = 8
N = 2048
D = 256
H = 8
DK = 32
QS = 256
H2 = 512
EPS = 1e-5
NCH = 16  # key chunks of 128

F32 = mybir.dt.float32
BF = mybir.dt.bfloat16
F8 = mybir.dt.float8e4

AF = mybir.ActivationFunctionType
ALU = mybir.AluOpType
AX = mybir.AxisListType

# Head processing order: pairs sharing a PE quadrant row offset (bp) share
# a psum zero region.
ORDER = [0, 4, 1, 5, 2, 6, 3, 7]


def build_kernel():
    nc = bacc.Bacc("TRN2", target_bir_lowering=False, debug=False,
                   num_devices=N_CORES)

    d_hT = nc.dram_tensor("hT", [D, N], BF, kind="ExternalInput")
    d_hTs = nc.dram_tensor("hTs", [D, QS], BF, kind="ExternalInput")
    d_hres = nc.dram_tensor("hres", [QS, D], BF, kind="ExternalInput")
    d_wq = nc.dram_tensor("wq", [D, D], BF, kind="ExternalInput")
    d_wk = nc.dram_tensor("wk", [D, D], BF, kind="ExternalInput")
    d_wv = nc.dram_tensor("wv", [D, D], BF, kind="ExternalInput")
    d_wo = nc.dram_tensor("wo", [D, D], BF, kind="ExternalInput")
    d_w1 = nc.dram_tensor("w1", [D, H2], BF, kind="ExternalInput")
    d_w2 = nc.dram_tensor("w2", [H2, D], BF, kind="ExternalInput")
    d_cols = nc.dram_tensor("cols", [128, 12], F32, kind="ExternalInput")
    d_bo = nc.dram_tensor("bo_eff", [1, D], BF, kind="ExternalInput")
    d_lng = nc.dram_tensor("lng", [128, 4 * D], BF, kind="ExternalInput")
    d_bias = nc.dram_tensor("bias8", [NCH, 128, H, QS], F8,
                            kind="ExternalInput")
    d_id8 = nc.dram_tensor("ident8", [128, 2, 128], F8, kind="ExternalInput")
    d_idb = nc.dram_tensor("identb", [128, 128], BF, kind="ExternalInput")
    d_out = nc.dram_tensor("out", [QS, D], F32, kind="ExternalOutput")

    with tile.TileContext(nc) as tc:
        import contextlib

        with contextlib.ExitStack() as ctx:
            wp = ctx.enter_context(tc.tile_pool(name="w", bufs=1))
            bpool = ctx.enter_context(tc.tile_pool(name="bias", bufs=5))
            ptp = ctx.enter_context(tc.tile_pool(name="pt", bufs=5))
            sm = ctx.enter_context(tc.tile_pool(name="sm", bufs=2))
            ps_sc = ctx.enter_context(
                tc.tile_pool(name="psc", bufs=2, space="PSUM"))
            ps_at = ctx.enter_context(
                tc.tile_pool(name="pat", bufs=1, space="PSUM"))
            ps_ms = ctx.enter_context(
                tc.tile_pool(name="pms", bufs=2, space="PSUM"))

            def load(pool, dram, shape, name, dt, r0=0, c0=0):
                t = pool.tile(shape, dt, name=name, tag=name)
                nc.sync.dma_start(
                    t[:], dram.ap()[r0:r0 + shape[0], c0:c0 + shape[1]])
                return t

            # ---- critical-path loads first ----
            hTs = [load(wp, d_hTs, [128, QS], f"hTs{i}", BF, 128 * i)
                   for i in range(2)]
            wq = [load(wp, d_wq, [128, D], f"wq{i}", BF, 128 * i)
                  for i in range(2)]
            cols = load(wp, d_cols, [128, 12], "cols", F32)
            id8 = wp.tile([128, 2, 128], F8, name="id8", tag="id8")
            nc.sync.dma_start(id8[:], d_id8.ap()[:, :, :])

            bias_tiles = {}

            def emit_bias_dma(c):
                t = bpool.tile([128, H, QS], F8, tag="bias", name=f"bias{c}")
                nc.sync.dma_start(t[:], d_bias.ap()[c])
                bias_tiles[c] = t

            # hT loaded in column pieces so K-proj can start early
            hT = [wp.tile([128, N], BF, name=f"hT{i}", tag=f"hT{i}")
                  for i in range(2)]

            def load_hT_piece(c0, c1):
                for i in range(2):
                    nc.sync.dma_start(
                        hT[i][:, c0:c1],
                        d_hT.ap()[128 * i:128 * i + 128, c0:c1])

            wk = [load(wp, d_wk, [128, D], f"wk{i}", BF, 128 * i)
                  for i in range(2)]
            load_hT_piece(0, 256)
            emit_bias_dma(0)
            load_hT_piece(256, 1024)
            wv = [load(wp, d_wv, [128, D], f"wv{i}", BF, 128 * i)
                  for i in range(2)]
            emit_bias_dma(1)
            load_hT_piece(1024, 2048)
            emit_bias_dma(2)

            bq = [cols[:, 0:1], cols[:, 1:2]]
            bk = [cols[:, 2:3], cols[:, 3:4]]
            b1c = [cols[:, 4 + i:5 + i] for i in range(4)]
            b2c = [cols[:, 8:9], cols[:, 9:10]]
            zcol = cols[:, 10:11]
            epscol = cols[:, 11:12]

            ones = wp.tile([1, 128], BF, name="ones", tag="ones")
            nc.vector.memset(ones[:], 1.0)

            # V with per-head all-ones denominator column (col 32 of 33)
            v_sb = wp.tile([128, NCH, H, 33], BF, name="v_sb", tag="v_sb")
            nc.vector.memset(v_sb[:, :, :, 32:33], 1.0)

            # ---------- Q projection ----------
            QT = []
            for oc in range(2):
                ps = ps_ms.tile([128, 512], F32, tag="pms", name="psq")
                for ic in range(2):
                    nc.tensor.matmul(
                        ps[:, 0:QS], wq[ic][:, 128 * oc:128 * oc + 128],
                        hTs[ic][:], start=(ic == 0), stop=(ic == 1))
                t = wp.tile([128, QS], BF, name=f"QT{oc}", tag=f"QT{oc}")
                nc.vector.tensor_scalar_add(t[:], ps[:, 0:QS], bq[oc])
                QT.append(t)

            KT = [wp.tile([128, N], BF, name=f"KT{i}", tag=f"KT{i}")
                  for i in range(2)]
            att = [ps_at.tile([128, H, 33], F32, tag=f"att{qt}",
                              name=f"att{qt}") for qt in range(2)]
            for qt in range(2):
                nc.vector.memset(att[qt][:], 0.0)
            pt_tiles = {}

            def emit_kproj(c0, c1):
                w = c1 - c0
                for oc in range(2):
                    ps = ps_ms.tile([128, 512], F32, tag="pms", name="psk")
                    for ic in range(2):
                        nc.tensor.matmul(
                            ps[:, 0:w], wk[ic][:, 128 * oc:128 * oc + 128],
                            hT[ic][:, c0:c1],
                            start=(ic == 0), stop=(ic == 1))
                    nc.vector.tensor_scalar_add(
                        KT[oc][:, c0:c1], ps[:, 0:w], bk[oc])

            def emit_vproj(c):
                ps = ps_ms.tile([128, 512], F32, tag="pms", name="psv")
                for ic in range(2):
                    nc.tensor.matmul(
                        ps[:, 0:D], hT[ic][:, 128 * c:128 * c + 128],
                        wv[ic][:], start=(ic == 0), stop=(ic == 1))
                nc.vector.tensor_copy(
                    v_sb[:, c, :, 0:32],
                    ps[:, 0:D].rearrange("p (h e) -> p h e", h=H))

            def emit_chunk(c):
                pt = ptp.tile([128, 2, 4, QS], BF, tag="pt", name=f"pt{c}")
                pt_tiles[c] = pt
                bt = bias_tiles.pop(c)
                for g in range(2):
                    ps = ps_sc.tile([128, 4, QS], F32, tag="sc",
                                    name=f"sc{c}_{g}")
                    for s in range(2):
                        rhs = bt[:, 4 * g + 2 * s:4 * g + 2 * s + 2, :]
                        rhs = rhs.rearrange("p a b -> p (a b)")
                        rhs = rhs.unsqueeze(1).broadcast_to((128, 2, 512))
                        nc.tensor.matmul(
                            ps[:, 2 * s:2 * s + 2, :], id8[:], rhs,
                            start=True, stop=False, skip_group_check=True,
                            perf_mode=mybir.MatmulPerfMode.DoubleRow)
                    for hh in range(4):
                        h = ORDER[4 * g + hh]
                        bp = 32 * (h % 4)
                        nc.tensor.matmul(
                            ps[:, hh, :],
                            KT[h // 4][bp:bp + 32, 128 * c:128 * c + 128],
                            QT[h // 4][bp:bp + 32, :],
                            start=False, stop=True, tile_position=(bp, 0),
                            skip_group_check=True)
                    nc.scalar.activation(pt[:, g], ps[:], AF.Exp, bias=zcol)

            def emit_attnv(c):
                pt = pt_tiles.pop(c)
                for qt in range(2):
                    for g in range(2):
                        for hh in range(4):
                            h = ORDER[4 * g + hh]
                            nc.tensor.matmul(
                                att[qt][:, h, :],
                                pt[:, g, hh, 128 * qt:128 * qt + 128],
                                v_sb[:, c, h, :],
                                start=False, stop=(c == NCH - 1),
                                skip_group_check=True)

            def emit_c(c):
                if c + 3 < NCH:
                    emit_bias_dma(c + 3)
                emit_vproj(c)
                emit_chunk(c)
                if c >= 2:
                    emit_attnv(c - 2)

            # ---------- main pipeline ----------
            emit_kproj(0, 256)
            emit_c(0)
            emit_kproj(256, 512)
            emit_c(1)
            emit_kproj(512, 1024)
            emit_c(2)
            emit_c(3)
            emit_kproj(1024, 1536)
            emit_c(4)
            emit_c(5)
            emit_kproj(1536, 2048)

            # deferred (epilogue-only) loads
            hres = [load(wp, d_hres, [128, D], f"hres{i}", BF, 128 * i)
                    for i in range(2)]
            wo = [load(wp, d_wo, [128, D], f"wo{i}", BF, 128 * i)
                  for i in range(2)]
            w1 = [load(wp, d_w1, [128, H2], f"w1{i}", BF, 128 * i)
                  for i in range(2)]
            w2 = [load(wp, d_w2, [128, D], f"w2{i}", BF, 128 * i)
                  for i in range(4)]
            bo = load(wp, d_bo, [1, D], "bo", BF)
            lng = load(wp, d_lng, [128, 4 * D], "lng", BF)
            idb = load(wp, d_idb, [128, 128], "idb", BF)

            for c in range(6, NCH):
                emit_c(c)
            emit_attnv(NCH - 2)
            emit_attnv(NCH - 1)

            # ---------- epilogue (stage-interleaved across qt) ----------
            o_nat = [wp.tile([128, D], BF, name=f"onat{qt}", tag=f"onat{qt}")
                     for qt in range(2)]
            OT = [wp.tile([128, D], BF, name=f"OT{fc}", tag=f"OT{fc}")
                  for fc in range(2)]
            fT = [wp.tile([128, D], BF, name=f"fT{fc}", tag=f"fT{fc}")
                  for fc in range(2)]
            g1T = [wp.tile([128, QS], BF, name=f"g1T{oc}", tag=f"g1T{oc}")
                   for oc in range(4)]
            y2T = [wp.tile([128, QS], BF, name=f"y2T{oc}", tag=f"y2T{oc}")
                   for oc in range(2)]
            h1 = [wp.tile([128, D], F32, name=f"h1_{qt}", tag=f"h1_{qt}")
                  for qt in range(2)]
            xin = [sm.tile([128, D], F32, tag=f"xin{qt}", name=f"xin{qt}")
                   for qt in range(2)]
            fln = [sm.tile([128, D], BF, tag=f"fln{qt}", name=f"fln{qt}")
                   for qt in range(2)]
            out_sb = [wp.tile([128, D], F32, name=f"osb{qt}", tag=f"osb{qt}")
                      for qt in range(2)]

            # normalize attention output (single broadcast multiply)
            for qt in range(2):
                rdt = sm.tile([128, H, 1], F32, tag=f"rd{qt}",
                              name=f"rd{qt}")
                nc.vector.reciprocal(rdt[:], att[qt][:, :, 32:33])
                nc.vector.tensor_mul(
                    o_nat[qt][:].rearrange("p (h e) -> p h e", h=H),
                    att[qt][:, :, 0:32],
                    rdt[:].broadcast_to((128, H, 32)))
            # transpose o_nat -> OT
            for qt in range(2):
                for fc in range(2):
                    tp = ps_ms.tile([128, 512], BF, tag="pms", name="trp")
                    nc.tensor.transpose(
                        tp[:, 0:128], o_nat[qt][:, 128 * fc:128 * fc + 128],
                        idb[:])
                    nc.vector.tensor_copy(
                        OT[fc][:, 128 * qt:128 * qt + 128], tp[:, 0:128])
            # out-projection + residual
            for qt in range(2):
                aps = ps_ms.tile([128, 512], F32, tag="pms", name="apo")
                for ic in range(2):
                    nc.tensor.matmul(
                        aps[:, 0:D], OT[ic][:, 128 * qt:128 * qt + 128],
                        wo[ic][:], start=(ic == 0), stop=False)
                nc.tensor.matmul(aps[:, 0:D], ones[:], bo[:],
                                 start=False, stop=True)
                nc.vector.tensor_add(xin[qt][:], aps[:, 0:D], hres[qt][:])

            def ln_stats(x_ap, tagp, eng):
                """Emit LN stats for one tile; returns (xc, st) where
                rstd is computed later (reciprocal of st)."""
                ss = sm.tile([128, 1], F32, tag=f"{tagp}ss", name="ss")
                nc.vector.reduce_sum(ss[:], x_ap, axis=AX.X)
                nm = sm.tile([128, 1], F32, tag=f"{tagp}nm", name="nm")
                eng.tensor_scalar_mul(nm[:], ss[:], -1.0 / D)
                xc = sm.tile([128, D], F32, tag=f"{tagp}xc", name="xc")
                eng.tensor_scalar_add(xc[:], x_ap, nm[:])
                scr = sm.tile([128, D], F32, tag=f"{tagp}scr", name="scr")
                eng.tensor_mul(scr[:], xc[:], xc[:])
                vs = sm.tile([128, 1], F32, tag=f"{tagp}vs", name="vs")
                nc.vector.reduce_sum(vs[:], scr[:], axis=AX.X)
                st = sm.tile([128, 1], F32, tag=f"{tagp}st", name="st")
                nc.scalar.activation(st[:], vs[:], AF.Sqrt, bias=epscol,
                                     scale=1.0 / D)
                return xc, st

            def ln_apply(xc, st, g_ap, b_ap, out_ap, tagp, eng):
                r0 = sm.tile([128, 1], F32, tag=f"{tagp}r0", name="r0")
                nc.vector.reciprocal(r0[:], st[:])
                if g_ap is None:  # gamma/beta folded into next matmul
                    eng.tensor_scalar_mul(out_ap, xc[:], r0[:])
                    return
                yp = sm.tile([128, D], F32, tag=f"{tagp}yp", name="yp")
                eng.tensor_scalar_mul(yp[:], xc[:], r0[:])
                yg = sm.tile([128, D], F32, tag=f"{tagp}yg", name="yg")
                eng.tensor_mul(yg[:], yp[:], g_ap)
                eng.tensor_add(out_ap, yg[:], b_ap)

            LNE = [nc.vector, nc.gpsimd]

            # LN1 both qt (Sqrt ops batched; qt1 runs on gpsimd)
            st1 = []
            for qt in range(2):
                st1.append(ln_stats(xin[qt][:], f"l1q{qt}", LNE[qt]))
            for qt in range(2):
                xc, st = st1[qt]
                ln_apply(xc, st, lng[:, 0:D], lng[:, D:2 * D], h1[qt][:],
                         f"l1q{qt}", LNE[qt])
            # LN2 both qt
            st2 = []
            for qt in range(2):
                st2.append(ln_stats(h1[qt][:], f"l2q{qt}", LNE[qt]))
            for qt in range(2):
                xc, st = st2[qt]
                ln_apply(xc, st, None, None,
                         fln[qt][:], f"l2q{qt}", LNE[qt])
            # transpose fln -> fT
            for qt in range(2):
                for fc in range(2):
                    tp2 = ps_ms.tile([128, 512], BF, tag="pms", name="trf")
                    nc.tensor.transpose(
                        tp2[:, 0:128], fln[qt][:, 128 * fc:128 * fc + 128],
                        idb[:])
                    nc.vector.tensor_copy(
                        fT[fc][:, 128 * qt:128 * qt + 128], tp2[:, 0:128])
            # FFN1 + gelu (gelus batched)
            for qt in range(2):
                for oc in range(4):
                    fp = ps_ms.tile([128, 512], F32, tag="pms", name="ps1")
                    for ic in range(2):
                        nc.tensor.matmul(
                            fp[:, 0:128],
                            w1[ic][:, 128 * oc:128 * oc + 128],
                            fT[ic][:, 128 * qt:128 * qt + 128],
                            start=(ic == 0), stop=(ic == 1))
                    nc.scalar.activation(
                        g1T[oc][:, 128 * qt:128 * qt + 128], fp[:, 0:128],
                        AF.Gelu, bias=b1c[oc])
            # FFN2 + transpose back + residual + store (per qt)
            for qt in range(2):
                for oc in range(2):
                    fp = ps_ms.tile([128, 512], F32, tag="pms", name="ps2")
                    for ic in range(4):
                        nc.tensor.matmul(
                            fp[:, 0:128],
                            w2[ic][:, 128 * oc:128 * oc + 128],
                            g1T[ic][:, 128 * qt:128 * qt + 128],
                            start=(ic == 0), stop=(ic == 3))
                    nc.vector.tensor_scalar_add(
                        y2T[oc][:, 128 * qt:128 * qt + 128], fp[:, 0:128],
                        b2c[oc])
                for fc in range(2):
                    tp3 = ps_ms.tile([128, 512], BF, tag="pms", name="trn")
                    nc.tensor.transpose(
                        tp3[:, 0:128], y2T[fc][:, 128 * qt:128 * qt + 128],
                        idb[:])
                    nc.vector.tensor_add(
                        out_sb[qt][:, 128 * fc:128 * fc + 128],
                        h1[qt][:, 128 * fc:128 * fc + 128], tp3[:, 0:128])
                nc.sync.dma_start(d_out.ap()[128 * qt:128 * qt + 128, :],
                                  out_sb[qt][:])

    nc.compile()
    return nc


_CACHE = {}
USE_FR = True


def _get_nc(use_fr=True):
    if use_fr not in _CACHE:
        _CACHE[use_fr] = build_kernel()
    return _CACHE[use_fr]


def kernel(**inputs):
    import ml_dtypes

    bf = ml_dtypes.bfloat16
    f8 = ml_dtypes.float8_e4m3fn

    h = np.asarray(inputs["h"], np.float32)
    edge_attr = np.asarray(inputs["edge_attr"], np.float32)
    edge_index = np.asarray(inputs["edge_index"])
    Wq, bq = np.asarray(inputs["Wq"], np.float32), np.asarray(inputs["bq"], np.float32)
    Wk, bk = np.asarray(inputs["Wk"], np.float32), np.asarray(inputs["bk"], np.float32)
    Wv, bv = np.asarray(inputs["Wv"], np.float32), np.asarray(inputs["bv"], np.float32)
    Wo, bo = np.asarray(inputs["Wo"], np.float32), np.asarray(inputs["bo"], np.float32)
    We, be = np.asarray(inputs["We"], np.float32), np.asarray(inputs["be"], np.float32)
    ln1_g, ln1_b = np.asarray(inputs["ln1_g"], np.float32), np.asarray(inputs["ln1_b"], np.float32)
    fln_g, fln_b = np.asarray(inputs["fln_g"], np.float32), np.asarray(inputs["fln_b"], np.float32)
    W1, b1 = np.asarray(inputs["W1"], np.float32), np.asarray(inputs["b1"], np.float32)
    W2, b2 = np.asarray(inputs["W2"], np.float32), np.asarray(inputs["b2"], np.float32)

    scale = 1.0 / np.sqrt(np.float32(DK))
    eb = edge_attr @ We + be  # (E, H)
    hT = np.ascontiguousarray(h.T)

    cols = np.zeros((128, 12), np.float32)
    cols[:, 0] = bq[0:128] * scale
    cols[:, 1] = bq[128:256] * scale
    cols[:, 2] = bk[0:128]
    cols[:, 3] = bk[128:256]
    b1_eff = b1 + fln_b @ W1
    for i in range(4):
        cols[:, 4 + i] = b1_eff[128 * i:128 * i + 128]
    cols[:, 8] = b2[0:128]
    cols[:, 9] = b2[128:256]
    cols[:, 11] = EPS

    bo_eff = (bv @ Wo + bo).reshape(1, D)
    lng = np.concatenate(
        [np.tile(x.reshape(1, D), (128, 1))
         for x in [ln1_g, ln1_b, fln_g, fln_b]], axis=1)

    common = {
        "hT": hT.astype(bf),
        "wq": (Wq * scale).astype(bf),
        "wk": Wk.astype(bf),
        "wv": Wv.astype(bf),
        "wo": Wo.astype(bf),
        "w1": (fln_g.reshape(D, 1) * W1).astype(bf),
        "w2": W2.astype(bf),
        "cols": cols,
        "bo_eff": bo_eff.astype(bf),
        "lng": lng.astype(bf),
        "ident8": np.stack(
            [np.eye(128, dtype=np.float32),
             np.zeros((128, 128), np.float32)], axis=1).astype(f8),
        "identb": np.eye(128, dtype=np.float32).astype(bf),
    }

    src = edge_index[0].astype(np.int64)
    dst = edge_index[1].astype(np.int64)
    in_maps = []
    for c in range(N_CORES):
        r0 = c * QS
        m = dict(common)
        m["hTs"] = np.ascontiguousarray(hT[:, r0:r0 + QS]).astype(bf)
        m["hres"] = np.ascontiguousarray(h[r0:r0 + QS]).astype(bf)
        sel = (src >= r0) & (src < r0 + QS)
        slab = np.zeros((N, H, QS), np.float32)
        slab[dst[sel], :, src[sel] - r0] = eb[sel]
        slab = slab[:, ORDER, :]  # head slot order
        m["bias8"] = np.ascontiguousarray(
            slab.reshape(NCH, 128, H, QS)).astype(f8)
        in_maps.append(m)

    nc = _get_nc(USE_FR)
    res = run_bass_kernel_spmd(nc, in_maps, core_ids=list(range(N_CORES)))
    out = np.concatenate([res.results[cc]["out"] for cc in range(N_CORES)],
                         axis=0)
    return out.astype(np.float32)
